# revision 1
# baseline (speedup 1.0000x reference)
"""Self-contained Trainium2 Bass kernel for nn_CharModel (dense transformer
forward: embed -> single-head causal attention -> vocab projection).

Distribution over 8 NeuronCores:
  - sequence-parallel attention: core c owns tokens [c*512, (c+1)*512)
  - vocab-parallel logits: core c owns padded-vocab columns [c*6400, (c+1)*6400)
  - attention outputs are exchanged with 4 chunked bf16 AllGathers
All matmuls run in bf16 with fp32 PSUM accumulation; softmax stats fp32.
"""
import numpy as np

import concourse.bass as bass
import concourse.mybir as mybir
import concourse.tile as tile
from concourse import bacc
from concourse.bass_utils import run_bass_kernel_spmd
from concourse.masks import make_identity

P = 128
N_TOK = 4096
D = 1024
VOCAB = 50257
NC = 8
VPAD_TOT = 51200  # 50257 padded up to 400*128
VSH = VPAD_TOT // NC  # 6400 per-core vocab shard
OWN = N_TOK // NC  # 512 own tokens
IBLK = OWN // P  # 4 own row-blocks
KT = D // P  # 8 contraction tiles
OT = D // P  # 8 output-feature tiles
CHUNKS = N_TOK // 512  # 8 projection chunks (512 tokens each)
JB = N_TOK // 512  # 8 key strips of 512
JB2 = N_TOK // P  # 32 key tiles of 128
SCALE = 1.0 / 32.0  # 1/sqrt(D)

F32 = mybir.dt.float32
F32R = mybir.dt.float32r
BF16 = mybir.dt.bfloat16
FP8 = mybir.dt.float8e4
I32 = mybir.dt.int32
WP_SCALE = 64.0
OUT_SCALE = 256.0

# logits v-strips within the 6400-wide shard: 12 x 512 + 1 x 256
VSTRIPS = [(i * 512, 512) for i in range(12)] + [(6144, 256)]


def build(nc: bass.Bass):
    tok = nc.dram_tensor("tok", [N_TOK], I32, kind="ExternalInput")
    qtok = nc.dram_tensor("qtok", [OWN], I32, kind="ExternalInput")
    E = nc.dram_tensor("E", [VOCAB, D], F32, kind="ExternalInput")
    WqT = nc.dram_tensor("WqT", [D, D], F32, kind="ExternalInput")
    WkT = nc.dram_tensor("WkT", [D, D], F32, kind="ExternalInput")
    WvT = nc.dram_tensor("WvT", [D, D], F32, kind="ExternalInput")
    bq = nc.dram_tensor("bq", [D], F32, kind="ExternalInput")
    bk = nc.dram_tensor("bk", [D], F32, kind="ExternalInput")
    bv = nc.dram_tensor("bv", [D], F32R, kind="ExternalInput")
    WpT = nc.dram_tensor("WpT", [D, VSH], F32R, kind="ExternalInput")
    bp = nc.dram_tensor("bp", [VSH], F32, kind="ExternalInput")
    # ridx_sh[r, jb] = global_row(r) - jb*512, fp32
    ridx_sh = nc.dram_tensor("ridx_sh", [OWN, JB], F32, kind="ExternalInput")
    logits = nc.dram_tensor("logits", [N_TOK, VSH], F32, kind="ExternalOutput")

    with tile.TileContext(nc) as tc:
        with (
            tc.tile_pool(name="const", bufs=1) as const,
            tc.tile_pool(name="dram", bufs=1, space="DRAM") as dram,
        ):
            ident = const.tile([P, P], BF16)
            make_identity(nc, ident[:])

            bv_t = const.tile([P, OT], F32R)
            nc.sync.dma_start(bv_t[:], bv.ap().rearrange("(ot p) -> p ot", p=P))

            bq_t = const.tile([P, OT], F32)
            nc.sync.dma_start(bq_t[:], bq.ap().rearrange("(ot p) -> p ot", p=P))
            bk_t = const.tile([P, OT], F32)
            nc.sync.dma_start(bk_t[:], bk.ap().rearrange("(ot p) -> p ot", p=P))

            rsh = const.tile([P, IBLK, JB], F32)
            nc.sync.dma_start(
                rsh[:], ridx_sh.ap().rearrange("(ib p) jb -> p ib jb", p=P)
            )

            jidx0 = const.tile([P, 512], F32)

            tok_sb = const.tile([P, N_TOK // P], I32)
            nc.sync.dma_start(tok_sb[:], tok.ap().rearrange("(g p) -> p g", p=P))
            qtok_sb = const.tile([P, OWN // P], I32)
            nc.sync.dma_start(qtok_sb[:], qtok.ap().rearrange("(g p) -> p g", p=P))

            # DRAM scratch
            Vscr = dram.tile([JB2, P, D], BF16)
            oTb = [dram.tile([P, KT, P], BF16, name=f"oTb{q}") for q in range(IBLK)]
            gat = [
                dram.tile([NC, P, KT, P], BF16, name=f"gat{q}") for q in range(IBLK)
            ]

            # ---------------- gather + transpose helper ----------------
            def gather_xT(pool, pspool, idx_sb, g0, ngroups, tag):
                """gather token groups [g0, g0+ngroups) -> xT [P, KT, ngroups*P] bf16"""
                xT = pool.tile([P, KT, ngroups * P], BF16, tag=f"xT_{tag}")
                for g in range(ngroups):
                    xg = pool.tile([P, D], F32, tag="xg")
                    nc.gpsimd.indirect_dma_start(
                        out=xg[:],
                        out_offset=None,
                        in_=E.ap(),
                        in_offset=bass.IndirectOffsetOnAxis(
                            ap=idx_sb[:, g0 + g : g0 + g + 1], axis=0
                        ),
                    )
                    xb = pool.tile([P, D], BF16, tag="xb")
                    nc.vector.tensor_copy(out=xb[:], in_=xg[:])
                    for kt in range(KT):
                        pst = pspool.tile([P, P], BF16, tag="ptr")
                        nc.tensor.transpose(
                            pst[:], xb[:, kt * P : (kt + 1) * P], ident[:]
                        )
                        nc.vector.tensor_copy(
                            out=xT[:, kt, g * P : (g + 1) * P], in_=pst[:]
                        )
                return xT

            def load_w(pool, dramt, tag):
                wb = pool.tile([P, KT, D], BF16, tag=f"wb_{tag}")
                for half in range(4):
                    wf = pool.tile([P, KT, D // 4], F32, tag="wf")
                    nc.sync.dma_start(
                        wf[:],
                        dramt.ap().rearrange("(kt p) o -> p kt o", p=P)[
                            :, :, half * (D // 4) : (half + 1) * (D // 4)
                        ],
                    )
                    nc.vector.tensor_copy(
                        out=wb[:, :, half * (D // 4) : (half + 1) * (D // 4)],
                        in_=wf[:],
                    )
                return wb

            # ---------------- phase Q: own-token Q projection ----------------
            qT_pool = tc.alloc_tile_pool(name="qT_keep", bufs=1)
            qT = qT_pool.tile([P, OT, OWN], BF16)
            kT_pool = tc.alloc_tile_pool(name="kT_keep", bufs=1)
            kT_all = kT_pool.tile([P, OT, N_TOK], BF16)
            with (
                tc.tile_pool(name="sbq", bufs=2) as sbq,
                tc.tile_pool(name="psq_tr", bufs=2, space="PSUM") as psq_tr,
                tc.tile_pool(name="psq_pp", bufs=4, space="PSUM") as psq_pp,
            ):
                ji = sbq.tile([P, 512], I32, tag="ji")
                nc.gpsimd.iota(ji[:], pattern=[[1, 512]], base=0, channel_multiplier=0)
                nc.vector.tensor_copy(out=jidx0[:], in_=ji[:])
                wq_b = load_w(sbq, WqT, "wq")
                xqT = gather_xT(sbq, psq_tr, qtok_sb, 0, OWN // P, "q")
                for ot in range(OT):
                    pp = psq_pp.tile([P, OWN], F32, tag="pp")
                    for kt in range(KT):
                        nc.tensor.matmul(
                            pp[:],
                            lhsT=wq_b[:, kt, ot * P : (ot + 1) * P],
                            rhs=xqT[:, kt, :],
                            start=(kt == 0),
                            stop=(kt == KT - 1),
                        )
                    nc.vector.tensor_scalar(
                        out=qT[:, ot, :],
                        in0=pp[:],
                        scalar1=bq_t[:, ot : ot + 1],
                        scalar2=SCALE,
                        op0=mybir.AluOpType.add,
                        op1=mybir.AluOpType.mult,
                    )

            # ---------------- phase KV: full K/V projections, spill to DRAM ----
            with (
                tc.tile_pool(name="sbkv", bufs=2) as sbkv,
                tc.tile_pool(name="pskv_tr", bufs=2, space="PSUM") as pskv_tr,
                tc.tile_pool(name="pskv_pp", bufs=2, space="PSUM") as pskv_pp,
                tc.tile_pool(name="pskv_pv", bufs=2, space="PSUM") as pskv_pv,
            ):
                wk_b = load_w(sbkv, WkT, "wk")
                wv_b = load_w(sbkv, WvT, "wv")
                for ch in range(CHUNKS):
                    xT = gather_xT(sbkv, pskv_tr, tok_sb, ch * 4, 4, "kv")
                    # K^T chunk -> Kscr[ch]
                    for ot in range(OT):
                        pk = pskv_pp.tile([P, 512], F32, tag="pp")
                        for kt in range(KT):
                            nc.tensor.matmul(
                                pk[:],
                                lhsT=wk_b[:, kt, ot * P : (ot + 1) * P],
                                rhs=xT[:, kt, :],
                                start=(kt == 0),
                                stop=(kt == KT - 1),
                            )
                        nc.vector.tensor_scalar(
                            out=kT_all[:, ot, ch * 512 : (ch + 1) * 512],
                            in0=pk[:],
                            scalar1=bk_t[:, ot : ot + 1],
                            scalar2=None,
                            op0=mybir.AluOpType.add,
                        )
                    # V natural chunk -> Vscr[ch*4 + tb]
                    for tb in range(4):
                        pv = pskv_pv.tile([P, D], F32, tag="pv")
                        for kt in range(KT):
                            nc.tensor.matmul(
                                pv[:, 0:512],
                                lhsT=xT[:, kt, tb * P : (tb + 1) * P],
                                rhs=wv_b[:, kt, 0:512],
                                start=(kt == 0),
                                stop=(kt == KT - 1),
                            )
                        for kt in range(KT):
                            nc.tensor.matmul(
                                pv[:, 512:1024],
                                lhsT=xT[:, kt, tb * P : (tb + 1) * P],
                                rhs=wv_b[:, kt, 512:1024],
                                start=(kt == 0),
                                stop=(kt == KT - 1),
                            )
                        ve = sbkv.tile([P, D], BF16, tag="ve")
                        nc.vector.tensor_copy(out=ve[:], in_=pv[:])
                        nc.sync.dma_start(Vscr[ch * 4 + tb, :, :], ve[:])

            # ---------------- phase attention (own rows) ----------------------
            # WpT load/cast pieces are interleaved between attention steps so
            # the Sync/Vector streams never block on a monolithic 25MB load.
            wp_pool = tc.alloc_tile_pool(name="wp_keep", bufs=1)
            wp_b = wp_pool.tile([P, KT, VSH], FP8)
            wp_pieces = [(kt, h) for kt in range(KT) for h in range(8)]
            WPW = VSH // 8  # 800-wide load/cast pieces

            bv_pool = tc.alloc_tile_pool(name="bv_keep", bufs=1)
            bvrow = bv_pool.tile([1, VSH], F32)

            with (
                tc.tile_pool(name="sbat", bufs=2) as sbat,
                tc.tile_pool(name="psat_sc", bufs=2, space="PSUM") as ps_sc,
                tc.tile_pool(name="psat_av", bufs=1, space="PSUM") as ps_av,
                tc.tile_pool(name="psat_tr", bufs=2, space="PSUM") as ps_tr,
                tc.tile_pool(name="psat_bv", bufs=2, space="PSUM") as ps_bv,
            ):
                nc.vector.memset(bvrow[:], 0.0)

                def load_wp_piece(i):
                    if i >= len(wp_pieces):
                        return
                    kt, half = wp_pieces[i]
                    v0 = half * WPW
                    v1 = (half + 1) * WPW
                    wpf = sbat.tile([P, WPW], F32R, tag="wpf")
                    nc.sync.dma_start(
                        wpf[:],
                        WpT.ap().rearrange("(kt p) v -> p kt v", p=P)[:, kt, v0:v1],
                    )
                    nc.vector.tensor_scalar(
                        out=wp_b[:, kt, v0:v1],
                        in0=wpf[:],
                        scalar1=WP_SCALE,
                        scalar2=None,
                        op0=mybir.AluOpType.mult,
                    )
                    # accumulate bv @ WpT into bvrow (fp32r matvec, 1 cyc/row)
                    for s0, sw in ((0, 512), (512, WPW - 512)):
                        pbv = ps_bv.tile([1, 512], F32, tag="bvp")
                        nc.tensor.matmul(
                            pbv[:, :sw],
                            lhsT=bv_t[:, kt : kt + 1],
                            rhs=wpf[:, s0 : s0 + sw],
                            start=True,
                            stop=True,
                        )
                        nc.vector.tensor_add(
                            out=bvrow[:, v0 + s0 : v0 + s0 + sw],
                            in0=bvrow[:, v0 + s0 : v0 + s0 + sw],
                            in1=pbv[:, :sw],
                        )

                wp_i = 0
                for ib in range(IBLK):
                    a_row = sbat.tile([P, N_TOK], BF16, tag="a_row")
                    for jb in range(JB):
                        load_wp_piece(wp_i)
                        load_wp_piece(wp_i + 1)
                        wp_i += 2
                        ps = ps_sc.tile([P, 512], F32, tag="sc")
                        for ot in range(OT):
                            nc.tensor.matmul(
                                ps[:],
                                lhsT=qT[:, ot, ib * P : (ib + 1) * P],
                                rhs=kT_all[:, ot, jb * 512 : (jb + 1) * 512],
                                start=(ot == 0),
                                stop=(ot == OT - 1),
                            )
                        astr = a_row[:, jb * 512 : (jb + 1) * 512]
                        nc.scalar.activation(
                            astr, ps[:], mybir.ActivationFunctionType.Exp
                        )
                        # multiply by causal mask: (jidx0 <= ridx - jb*512) * exp
                        nc.vector.scalar_tensor_tensor(
                            out=astr,
                            in0=jidx0[:],
                            scalar=rsh[:, ib, jb : jb + 1],
                            in1=astr,
                            op0=mybir.AluOpType.is_le,
                            op1=mybir.AluOpType.mult,
                        )
                    dsum = sbat.tile([P, 1], F32, tag="dsum")
                    nc.vector.tensor_reduce(
                        out=dsum[:],
                        in_=a_row[:],
                        axis=mybir.AxisListType.X,
                        op=mybir.AluOpType.add,
                    )
                    rden = sbat.tile([P, 1], F32, tag="rden")
                    nc.vector.reciprocal(rden[:], dsum[:])

                    pav = ps_av.tile([P, D], F32, tag="av")
                    for j2 in range(JB2):
                        pat = ps_tr.tile([P, P], BF16, tag="tr")
                        nc.tensor.transpose(
                            pat[:], a_row[:, j2 * P : (j2 + 1) * P], ident[:]
                        )
                        at = sbat.tile([P, P], BF16, tag="at")
                        nc.vector.tensor_copy(out=at[:], in_=pat[:])
                        vj = sbat.tile([P, D], BF16, tag="vj")
                        nc.sync.dma_start(vj[:], Vscr[j2, :, :])
                        nc.tensor.matmul(
                            pav[:, 0:512],
                            lhsT=at[:],
                            rhs=vj[:, 0:512],
                            start=(j2 == 0),
                            stop=(j2 == JB2 - 1),
                        )
                        nc.tensor.matmul(
                            pav[:, 512:1024],
                            lhsT=at[:],
                            rhs=vj[:, 512:1024],
                            start=(j2 == 0),
                            stop=(j2 == JB2 - 1),
                        )
                    o_bf = sbat.tile([P, D], BF16, tag="o_bf")
                    nc.vector.tensor_scalar(
                        out=o_bf[:],
                        in0=pav[:],
                        scalar1=rden[:, :1],
                        scalar2=None,
                        op0=mybir.AluOpType.mult,
                    )
                    oT = sbat.tile([P, KT, P], BF16, tag="oT")
                    for kt in range(KT):
                        pot = ps_tr.tile([P, P], BF16, tag="tr")
                        nc.tensor.transpose(
                            pot[:], o_bf[:, kt * P : (kt + 1) * P], ident[:]
                        )
                        nc.vector.tensor_copy(out=oT[:, kt, :], in_=pot[:])
                    nc.sync.dma_start(oTb[ib][:], oT[:])
                    nc.gpsimd.collective_compute(
                        "AllGather",
                        mybir.AluOpType.bypass,
                        replica_groups=[list(range(NC))],
                        ins=[oTb[ib].opt()],
                        outs=[gat[ib].opt()],
                    )

            # ---------------- phase logits ------------------------------------
            with (
                tc.tile_pool(name="sblg", bufs=2) as sblg,
                tc.tile_pool(name="sbbp", bufs=1) as sbbp,
                tc.tile_pool(name="pslg", bufs=6, space="PSUM") as pslg,
            ):
                # bvrow += bp (piecewise), spill to DRAM, broadcast back
                for h in range(8):
                    bpp = sblg.tile([1, WPW], F32, tag="bpp")
                    nc.sync.dma_start(
                        bpp[:], bp.ap()[None, h * WPW : (h + 1) * WPW]
                    )
                    nc.vector.tensor_add(
                        out=bvrow[:, h * WPW : (h + 1) * WPW],
                        in0=bvrow[:, h * WPW : (h + 1) * WPW],
                        in1=bpp[:],
                    )
                bpx = dram.tile([VSH], F32, name="bpx")
                nc.sync.dma_start(bpx[:][None, :], bvrow[:])
                bp_bc = sbbp.tile([P, VSH], F32)
                nc.sync.dma_start(bp_bc[:], bpx[:][None, :].to_broadcast([P, VSH]))
                for q in range(IBLK):
                    for c in range(NC):
                        ibg = c * IBLK + q  # global row-block
                        lt = sblg.tile([P, KT, P], BF16, tag="lt")
                        nc.sync.dma_start(lt[:], gat[q][c, :, :, :])
                        lt8 = sblg.tile([P, KT, P], FP8, tag="lt8")
                        nc.vector.tensor_scalar(
                            out=lt8[:],
                            in0=lt[:],
                            scalar1=OUT_SCALE,
                            scalar2=None,
                            op0=mybir.AluOpType.mult,
                        )
                        for v0, vw in VSTRIPS:
                            pl = pslg.tile([P, 512], F32, tag="lg")
                            for k2 in range(KT // 2):
                                nc.tensor.matmul(
                                    pl[:, :vw],
                                    lhsT=lt8[:, 2 * k2 : 2 * k2 + 2, :],
                                    rhs=wp_b[:, 2 * k2 : 2 * k2 + 2, v0 : v0 + vw],
                                    start=(k2 == 0),
                                    stop=(k2 == KT // 2 - 1),
                                    perf_mode=mybir.MatmulPerfMode.DoubleRow,
                                )
                            lo = sblg.tile([P, 512], F32, tag="lo")
                            nc.vector.scalar_tensor_tensor(
                                out=lo[:, :vw],
                                in0=pl[:, :vw],
                                scalar=1.0 / (WP_SCALE * OUT_SCALE),
                                in1=bp_bc[:, v0 : v0 + vw],
                                op0=mybir.AluOpType.mult,
                                op1=mybir.AluOpType.add,
                            )
                            nc.sync.dma_start(
                                logits.ap()[
                                    ibg * P : (ibg + 1) * P, v0 : v0 + vw
                                ],
                                lo[:, :vw],
                            )
            bv_pool.release()
            wp_pool.release()
            kT_pool.release()
            qT_pool.release()
    return nc


def _prep_inputs(inputs):
    """Host-side shard prep: slicing, transposes, padding only."""
    tokens = np.ascontiguousarray(np.asarray(inputs["tokens"]).astype(np.int32))
    E = np.asarray(inputs["E"], dtype=np.float32)
    WqT = np.ascontiguousarray(np.asarray(inputs["Wq"], np.float32).T)
    WkT = np.ascontiguousarray(np.asarray(inputs["Wk"], np.float32).T)
    WvT = np.ascontiguousarray(np.asarray(inputs["Wv"], np.float32).T)
    Wp = np.asarray(inputs["Wp"], np.float32)
    WpT_pad = np.zeros((D, VPAD_TOT), np.float32)
    WpT_pad[:, :VOCAB] = Wp.T
    bp_pad = np.zeros((VPAD_TOT,), np.float32)
    bp_pad[:VOCAB] = np.asarray(inputs["bp"], np.float32)

    in_maps = []
    for c in range(NC):
        rows = np.arange(c * OWN, (c + 1) * OWN, dtype=np.float32)
        ridx_sh = rows[:, None] - 512.0 * np.arange(JB, dtype=np.float32)[None, :]
        in_maps.append(
            {
                "tok": tokens,
                "qtok": np.ascontiguousarray(tokens[c * OWN : (c + 1) * OWN]),
                "E": E,
                "WqT": WqT,
                "WkT": WkT,
                "WvT": WvT,
                "bq": np.asarray(inputs["bq"], np.float32),
                "bk": np.asarray(inputs["bk"], np.float32),
                "bv": np.asarray(inputs["bv"], np.float32),
                "WpT": np.ascontiguousarray(WpT_pad[:, c * VSH : (c + 1) * VSH]),
                "bp": np.ascontiguousarray(bp_pad[c * VSH : (c + 1) * VSH]),
                "ridx_sh": np.ascontiguousarray(ridx_sh, dtype=np.float32),
            }
        )
    return in_maps


def _run(inputs, trace=False):
    nc = bacc.Bacc(trn_type="TRN2", num_devices=NC, debug=False)
    build(nc)
    nc.compile()
    in_maps = _prep_inputs(inputs)
    res = run_bass_kernel_spmd(
        nc, in_maps, core_ids=list(range(NC)), trace=trace
    )
    out = np.concatenate(
        [res.results[c]["logits"] for c in range(NC)], axis=1
    )[:, :VOCAB]
    return out, res


def kernel(**inputs) -> np.ndarray:
    out, _ = _run(inputs, trace=False)
    return out



# revision 4
# speedup vs baseline: 1.2830x; 1.2830x over previous
"""Self-contained Trainium2 Bass kernel for nn_CharModel (dense transformer
forward: embed -> single-head causal attention -> vocab projection).

Distribution over 8 NeuronCores:
  - sequence-parallel QKV: core c computes Q/K/V only for its own 512 tokens,
    K^T and V are exchanged with one bf16 AllGather (2MB/rank)
  - sequence-parallel attention rows, vocab-parallel logits (6400 cols/core)
  - attention outputs exchanged with 4 chunked bf16 AllGathers
Host pre-casts: E/Wq/Wk/Wv to bf16, Wp.T*64 to fp8e4, and folds bv@Wp.T+bp
into a single f32 bias row. Logits are written bf16 (host upcasts).
"""
import numpy as np
import ml_dtypes

import concourse.bass as bass
import concourse.mybir as mybir
import concourse.tile as tile
from concourse import bacc
from concourse.bass_utils import run_bass_kernel_spmd
from concourse.masks import make_identity

P = 128
N_TOK = 4096
D = 1024
VOCAB = 50257
NC = 8
VPAD_TOT = 51200  # 50257 padded up to 400*128
VSH = VPAD_TOT // NC  # 6400 per-core vocab shard
OWN = N_TOK // NC  # 512 own tokens
IBLK = OWN // P  # 4 own row-blocks
KT = D // P  # 8 contraction tiles
OT = D // P  # 8 output-feature tiles
JB = N_TOK // 512  # 8 key strips of 512
JB2 = N_TOK // P  # 32 key tiles of 128
SCALE = 1.0 / 32.0  # 1/sqrt(D)

F32 = mybir.dt.float32
BF16 = mybir.dt.bfloat16
FP8 = mybir.dt.float8e4
I32 = mybir.dt.int32
WP_SCALE = 64.0
OUT_SCALE = 256.0

KV_K = P * OT * 512  # kT section elements in the kv exchange buffer
KV_V = P * D  # one V tile (128 tokens x 1024)
KV_ELEMS = KV_K + IBLK * KV_V

# logits v-strips within the 6400-wide shard: 12 x 512 + 1 x 256
VSTRIPS = [(i * 512, 512) for i in range(12)] + [(6144, 256)]


def build(nc: bass.Bass):
    qtok = nc.dram_tensor("qtok", [OWN], I32, kind="ExternalInput")
    E = nc.dram_tensor("E", [VOCAB, D], BF16, kind="ExternalInput")
    WqT = nc.dram_tensor("WqT", [D, D], BF16, kind="ExternalInput")
    WkT = nc.dram_tensor("WkT", [D, D], BF16, kind="ExternalInput")
    WvT = nc.dram_tensor("WvT", [D, D], BF16, kind="ExternalInput")
    bq = nc.dram_tensor("bq", [D], F32, kind="ExternalInput")
    bk = nc.dram_tensor("bk", [D], F32, kind="ExternalInput")
    Wp8 = nc.dram_tensor("Wp8", [D, VSH], FP8, kind="ExternalInput")
    bias_row = nc.dram_tensor("bias_row", [VSH], F32, kind="ExternalInput")
    # ridx_sh[r, jb] = global_row(r) - jb*512, fp32
    ridx_sh = nc.dram_tensor("ridx_sh", [OWN, JB], F32, kind="ExternalInput")
    logits = nc.dram_tensor("logits", [N_TOK, VSH], BF16, kind="ExternalOutput")

    with tile.TileContext(nc) as tc:
        with (
            tc.tile_pool(name="const", bufs=1) as const,
            tc.tile_pool(name="dram", bufs=1, space="DRAM") as dram,
        ):
            ident = const.tile([P, P], BF16)
            make_identity(nc, ident[:])

            bq_t = const.tile([P, OT], F32)
            nc.sync.dma_start(bq_t[:], bq.ap().rearrange("(ot p) -> p ot", p=P))
            bk_t = const.tile([P, OT], F32)
            nc.sync.dma_start(bk_t[:], bk.ap().rearrange("(ot p) -> p ot", p=P))

            rsh = const.tile([P, IBLK, JB], F32)
            nc.sync.dma_start(
                rsh[:], ridx_sh.ap().rearrange("(ib p) jb -> p ib jb", p=P)
            )

            jidx0 = const.tile([P, 512], F32)

            qtok_sb = const.tile([P, OWN // P], I32)
            nc.sync.dma_start(qtok_sb[:], qtok.ap().rearrange("(g p) -> p g", p=P))

            # DRAM scratch for collectives
            kv_send = dram.tile([KV_ELEMS], BF16)
            kvg = dram.tile([NC, KV_ELEMS], BF16, addr_space="Shared")
            oTb = [dram.tile([P, KT, P], BF16, name=f"oTb{q}") for q in range(IBLK)]
            gat = [
                dram.tile(
                    [NC, P, KT, P], BF16, name=f"gat{q}", addr_space="Shared"
                )
                for q in range(IBLK)
            ]

            # ---------------- phase QKV: own tokens only --------------------
            qT_pool = tc.alloc_tile_pool(name="qT_keep", bufs=1)
            qT = qT_pool.tile([P, OT, OWN], BF16)
            kT_pool = tc.alloc_tile_pool(name="kT_keep", bufs=1)
            kT_all = kT_pool.tile([P, OT, N_TOK], BF16)
            with (
                tc.tile_pool(name="sbw", bufs=1) as sbw,
                tc.tile_pool(name="sbq", bufs=2) as sbq,
                tc.tile_pool(name="psq_tr", bufs=2, space="PSUM") as psq_tr,
                tc.tile_pool(name="psq_pp", bufs=2, space="PSUM") as psq_pp,
                tc.tile_pool(name="psq_pv", bufs=2, space="PSUM") as psq_pv,
            ):
                ji = sbw.tile([P, 512], I32, tag="ji")
                nc.gpsimd.iota(ji[:], pattern=[[1, 512]], base=0, channel_multiplier=0)
                nc.vector.tensor_copy(out=jidx0[:], in_=ji[:])

                wq_b = sbw.tile([P, KT, D], BF16, tag="wq")
                nc.sync.dma_start(
                    wq_b[:], WqT.ap().rearrange("(kt p) o -> p kt o", p=P)
                )
                wk_b = sbw.tile([P, KT, D], BF16, tag="wk")
                nc.sync.dma_start(
                    wk_b[:], WkT.ap().rearrange("(kt p) o -> p kt o", p=P)
                )
                wv_b = sbw.tile([P, KT, D], BF16, tag="wv")
                nc.sync.dma_start(
                    wv_b[:], WvT.ap().rearrange("(kt p) o -> p kt o", p=P)
                )

                # gather own embeddings + transpose -> xT [P, KT, OWN]
                xT = sbw.tile([P, KT, OWN], BF16, tag="xT")
                for g in range(IBLK):
                    xg = sbq.tile([P, D], BF16, tag="xg")
                    nc.gpsimd.indirect_dma_start(
                        out=xg[:],
                        out_offset=None,
                        in_=E.ap(),
                        in_offset=bass.IndirectOffsetOnAxis(
                            ap=qtok_sb[:, g : g + 1], axis=0
                        ),
                    )
                    for kt in range(KT):
                        pst = psq_tr.tile([P, P], BF16, tag="ptr")
                        nc.tensor.transpose(
                            pst[:], xg[:, kt * P : (kt + 1) * P], ident[:]
                        )
                        nc.vector.tensor_copy(
                            out=xT[:, kt, g * P : (g + 1) * P], in_=pst[:]
                        )

                # Q^T (scaled by 1/sqrt(d)) kept in SBUF
                for ot in range(OT):
                    pp = psq_pp.tile([P, OWN], F32, tag="pp")
                    for kt in range(KT):
                        nc.tensor.matmul(
                            pp[:],
                            lhsT=wq_b[:, kt, ot * P : (ot + 1) * P],
                            rhs=xT[:, kt, :],
                            start=(kt == 0),
                            stop=(kt == KT - 1),
                        )
                    nc.vector.tensor_scalar(
                        out=qT[:, ot, :],
                        in0=pp[:],
                        scalar1=bq_t[:, ot : ot + 1],
                        scalar2=SCALE,
                        op0=mybir.AluOpType.add,
                        op1=mybir.AluOpType.mult,
                    )

                # K^T for own tokens -> kv_send[0:KV_K]
                kT_own = sbw.tile([P, OT, OWN], BF16, tag="kT_own")
                for ot in range(OT):
                    pk = psq_pp.tile([P, OWN], F32, tag="pp")
                    for kt in range(KT):
                        nc.tensor.matmul(
                            pk[:],
                            lhsT=wk_b[:, kt, ot * P : (ot + 1) * P],
                            rhs=xT[:, kt, :],
                            start=(kt == 0),
                            stop=(kt == KT - 1),
                        )
                    nc.vector.tensor_scalar(
                        out=kT_own[:, ot, :],
                        in0=pk[:],
                        scalar1=bk_t[:, ot : ot + 1],
                        scalar2=None,
                        op0=mybir.AluOpType.add,
                    )
                nc.sync.dma_start(
                    kv_send[0:KV_K].rearrange(
                        "(p ot j) -> p ot j", p=P, ot=OT
                    ),
                    kT_own[:],
                )

                # V for own tokens (NO bias - folded into host bias_row)
                for tb in range(IBLK):
                    pv = psq_pv.tile([P, D], F32, tag="pv")
                    for half in range(2):
                        for kt in range(KT):
                            nc.tensor.matmul(
                                pv[:, half * 512 : (half + 1) * 512],
                                lhsT=xT[:, kt, tb * P : (tb + 1) * P],
                                rhs=wv_b[:, kt, half * 512 : (half + 1) * 512],
                                start=(kt == 0),
                                stop=(kt == KT - 1),
                            )
                    ve = sbq.tile([P, D], BF16, tag="ve")
                    nc.vector.tensor_copy(out=ve[:], in_=pv[:])
                    nc.sync.dma_start(
                        kv_send[
                            KV_K + tb * KV_V : KV_K + (tb + 1) * KV_V
                        ].rearrange("(p d) -> p d", p=P),
                        ve[:],
                    )

                nc.gpsimd.collective_compute(
                    "AllGather",
                    mybir.AluOpType.bypass,
                    replica_groups=[list(range(NC))],
                    ins=[kv_send.opt()],
                    outs=[kvg.opt()],
                )

            # ---------------- load gathered K^T + Wp + bias -----------------
            wp_pool = tc.alloc_tile_pool(name="wp_keep", bufs=1)
            wp_b = wp_pool.tile([P, KT, VSH], FP8)
            nc.sync.dma_start(
                wp_b[:], Wp8.ap().rearrange("(kt p) v -> p kt v", p=P)
            )
            bp_pool = tc.alloc_tile_pool(name="bp_keep", bufs=1)
            bp_bc = bp_pool.tile([P, VSH], F32)
            nc.sync.dma_start(
                bp_bc[:], bias_row.ap()[None, :].to_broadcast([P, VSH])
            )
            for r in range(NC):
                nc.sync.dma_start(
                    kT_all[:, :, r * 512 : (r + 1) * 512],
                    kvg[r, 0:KV_K].rearrange("(p ot j) -> p ot j", p=P, ot=OT),
                )

            # ---------------- phase attention (own rows) --------------------
            with (
                tc.tile_pool(name="sbat", bufs=2) as sbat,
                tc.tile_pool(name="psat_sc", bufs=2, space="PSUM") as ps_sc,
                tc.tile_pool(name="psat_av", bufs=1, space="PSUM") as ps_av,
                tc.tile_pool(name="psat_tr", bufs=2, space="PSUM") as ps_tr,
            ):
                for ib in range(IBLK):
                    a_row = sbat.tile([P, N_TOK], BF16, tag="a_row")
                    for jb in range(JB):
                        ps = ps_sc.tile([P, 512], F32, tag="sc")
                        for ot in range(OT):
                            nc.tensor.matmul(
                                ps[:],
                                lhsT=qT[:, ot, ib * P : (ib + 1) * P],
                                rhs=kT_all[:, ot, jb * 512 : (jb + 1) * 512],
                                start=(ot == 0),
                                stop=(ot == OT - 1),
                            )
                        astr = a_row[:, jb * 512 : (jb + 1) * 512]
                        nc.scalar.activation(
                            astr, ps[:], mybir.ActivationFunctionType.Exp
                        )
                        # multiply by causal mask: (jidx0 <= ridx - jb*512) * exp
                        nc.vector.scalar_tensor_tensor(
                            out=astr,
                            in0=jidx0[:],
                            scalar=rsh[:, ib, jb : jb + 1],
                            in1=astr,
                            op0=mybir.AluOpType.is_le,
                            op1=mybir.AluOpType.mult,
                        )
                    dsum = sbat.tile([P, 1], F32, tag="dsum")
                    nc.vector.tensor_reduce(
                        out=dsum[:],
                        in_=a_row[:],
                        axis=mybir.AxisListType.X,
                        op=mybir.AluOpType.add,
                    )
                    rden = sbat.tile([P, 1], F32, tag="rden")
                    nc.vector.reciprocal(rden[:], dsum[:])

                    pav = ps_av.tile([P, D], F32, tag="av")
                    for j2 in range(JB2):
                        pat = ps_tr.tile([P, P], BF16, tag="tr")
                        nc.tensor.transpose(
                            pat[:], a_row[:, j2 * P : (j2 + 1) * P], ident[:]
                        )
                        at = sbat.tile([P, P], BF16, tag="at")
                        nc.vector.tensor_copy(out=at[:], in_=pat[:])
                        vj = sbat.tile([P, D], BF16, tag="vj")
                        r, tb = j2 // IBLK, j2 % IBLK
                        nc.sync.dma_start(
                            vj[:],
                            kvg[
                                r, KV_K + tb * KV_V : KV_K + (tb + 1) * KV_V
                            ].rearrange("(p d) -> p d", p=P),
                        )
                        nc.tensor.matmul(
                            pav[:, 0:512],
                            lhsT=at[:],
                            rhs=vj[:, 0:512],
                            start=(j2 == 0),
                            stop=(j2 == JB2 - 1),
                        )
                        nc.tensor.matmul(
                            pav[:, 512:1024],
                            lhsT=at[:],
                            rhs=vj[:, 512:1024],
                            start=(j2 == 0),
                            stop=(j2 == JB2 - 1),
                        )
                    o_bf = sbat.tile([P, D], BF16, tag="o_bf")
                    nc.vector.tensor_scalar(
                        out=o_bf[:],
                        in0=pav[:],
                        scalar1=rden[:, :1],
                        scalar2=None,
                        op0=mybir.AluOpType.mult,
                    )
                    oT = sbat.tile([P, KT, P], BF16, tag="oT")
                    for kt in range(KT):
                        pot = ps_tr.tile([P, P], BF16, tag="tr")
                        nc.tensor.transpose(
                            pot[:], o_bf[:, kt * P : (kt + 1) * P], ident[:]
                        )
                        nc.vector.tensor_copy(out=oT[:, kt, :], in_=pot[:])
                    nc.sync.dma_start(oTb[ib][:], oT[:])
                    nc.gpsimd.collective_compute(
                        "AllGather",
                        mybir.AluOpType.bypass,
                        replica_groups=[list(range(NC))],
                        ins=[oTb[ib].opt()],
                        outs=[gat[ib].opt()],
                    )

            # ---------------- phase logits ----------------------------------
            with (
                tc.tile_pool(name="sblg", bufs=2) as sblg,
                tc.tile_pool(name="pslg", bufs=6, space="PSUM") as pslg,
            ):
                for q in range(IBLK):
                    for c in range(NC):
                        ibg = c * IBLK + q  # global row-block
                        lt = sblg.tile([P, KT, P], BF16, tag="lt")
                        nc.sync.dma_start(lt[:], gat[q][c, :, :, :])
                        lt8 = sblg.tile([P, KT, P], FP8, tag="lt8")
                        nc.vector.tensor_scalar(
                            out=lt8[:],
                            in0=lt[:],
                            scalar1=OUT_SCALE,
                            scalar2=None,
                            op0=mybir.AluOpType.mult,
                        )
                        for v0, vw in VSTRIPS:
                            pl = pslg.tile([P, 512], F32, tag="lg")
                            for k2 in range(KT // 2):
                                nc.tensor.matmul(
                                    pl[:, :vw],
                                    lhsT=lt8[:, 2 * k2 : 2 * k2 + 2, :],
                                    rhs=wp_b[:, 2 * k2 : 2 * k2 + 2, v0 : v0 + vw],
                                    start=(k2 == 0),
                                    stop=(k2 == KT // 2 - 1),
                                    perf_mode=mybir.MatmulPerfMode.DoubleRow,
                                )
                            lo = sblg.tile([P, 512], BF16, tag="lo")
                            nc.vector.scalar_tensor_tensor(
                                out=lo[:, :vw],
                                in0=pl[:, :vw],
                                scalar=1.0 / (WP_SCALE * OUT_SCALE),
                                in1=bp_bc[:, v0 : v0 + vw],
                                op0=mybir.AluOpType.mult,
                                op1=mybir.AluOpType.add,
                            )
                            nc.sync.dma_start(
                                logits.ap()[
                                    ibg * P : (ibg + 1) * P, v0 : v0 + vw
                                ],
                                lo[:, :vw],
                            )
            bp_pool.release()
            wp_pool.release()
            kT_pool.release()
            qT_pool.release()
    return nc


def _prep_inputs(inputs):
    """Host-side shard prep: slicing, transposes, padding, dtype pre-casts."""
    tokens = np.ascontiguousarray(np.asarray(inputs["tokens"]).astype(np.int32))
    E16 = np.asarray(inputs["E"], np.float32).astype(ml_dtypes.bfloat16)
    WqT = np.ascontiguousarray(
        np.asarray(inputs["Wq"], np.float32).T.astype(ml_dtypes.bfloat16)
    )
    WkT = np.ascontiguousarray(
        np.asarray(inputs["Wk"], np.float32).T.astype(ml_dtypes.bfloat16)
    )
    WvT = np.ascontiguousarray(
        np.asarray(inputs["Wv"], np.float32).T.astype(ml_dtypes.bfloat16)
    )
    Wp = np.asarray(inputs["Wp"], np.float32)
    bv = np.asarray(inputs["bv"], np.float32)
    WpT_pad = np.zeros((D, VPAD_TOT), np.float32)
    WpT_pad[:, :VOCAB] = Wp.T
    Wp8_full = (WpT_pad * WP_SCALE).astype(ml_dtypes.float8_e4m3)
    bias_full = np.zeros((VPAD_TOT,), np.float32)
    bias_full[:VOCAB] = np.asarray(inputs["bp"], np.float32) + Wp @ bv

    in_maps = []
    for c in range(NC):
        rows = np.arange(c * OWN, (c + 1) * OWN, dtype=np.float32)
        ridx_sh = rows[:, None] - 512.0 * np.arange(JB, dtype=np.float32)[None, :]
        in_maps.append(
            {
                "qtok": np.ascontiguousarray(tokens[c * OWN : (c + 1) * OWN]),
                "E": E16,
                "WqT": WqT,
                "WkT": WkT,
                "WvT": WvT,
                "bq": np.asarray(inputs["bq"], np.float32),
                "bk": np.asarray(inputs["bk"], np.float32),
                "Wp8": np.ascontiguousarray(Wp8_full[:, c * VSH : (c + 1) * VSH]),
                "bias_row": np.ascontiguousarray(
                    bias_full[c * VSH : (c + 1) * VSH]
                ),
                "ridx_sh": np.ascontiguousarray(ridx_sh, dtype=np.float32),
            }
        )
    return in_maps


def _run(inputs, trace=False):
    nc = bacc.Bacc(trn_type="TRN2", num_devices=NC, debug=False)
    build(nc)
    nc.compile()
    in_maps = _prep_inputs(inputs)
    res = run_bass_kernel_spmd(
        nc, in_maps, core_ids=list(range(NC)), trace=trace
    )
    out = np.concatenate(
        [np.asarray(res.results[c]["logits"], np.float32) for c in range(NC)],
        axis=1,
    )[:, :VOCAB]
    return out, res


def kernel(**inputs) -> np.ndarray:
    out, _ = _run(inputs, trace=False)
    return out


# revision 6
# speedup vs baseline: 1.4502x; 1.1304x over previous
"""Self-contained Trainium2 Bass kernel for nn_CharModel (dense transformer
forward: embed -> single-head causal attention -> vocab projection).

Distribution over 8 NeuronCores:
  - sequence-parallel QKV: core c computes Q/K/V only for its own 512 tokens,
    K^T and V are exchanged with one bf16 AllGather (2MB/rank)
  - sequence-parallel attention rows, vocab-parallel logits (6400 cols/core)
  - attention outputs exchanged with 4 chunked bf16 AllGathers
Host pre-casts: E/Wq/Wk/Wv to bf16, Wp.T*64 to fp8e4, and folds bv@Wp.T+bp
into a single f32 bias row. Logits are written bf16 (host upcasts).
"""
import numpy as np
import ml_dtypes

import concourse.bass as bass
import concourse.mybir as mybir
import concourse.tile as tile
from concourse import bacc
from concourse.bass_utils import run_bass_kernel_spmd
from concourse.masks import make_identity

P = 128
N_TOK = 4096
D = 1024
VOCAB = 50257
NC = 8
VPAD_TOT = 51200  # 50257 padded up to 400*128
VSH = VPAD_TOT // NC  # 6400 per-core vocab shard
OWN = N_TOK // NC  # 512 own tokens
IBLK = OWN // P  # 4 own row-blocks
KT = D // P  # 8 contraction tiles
OT = D // P  # 8 output-feature tiles
JB = N_TOK // 512  # 8 key strips of 512
JB2 = N_TOK // P  # 32 key tiles of 128
SCALE = 1.0 / 32.0  # 1/sqrt(D)

F32 = mybir.dt.float32
BF16 = mybir.dt.bfloat16
FP8 = mybir.dt.float8e4
I32 = mybir.dt.int32
WP_SCALE = 64.0
OUT_SCALE = 256.0

KV_K = P * OT * 512  # kT section elements in the kv exchange buffer
KV_V = P * D  # one V tile (128 tokens x 1024)
KV_ELEMS = KV_K + IBLK * KV_V

# logits v-strips within the 6400-wide shard: 12 x 512 + 1 x 256
VSTRIPS = [(i * 512, 512) for i in range(12)] + [(6144, 256)]


def build(nc: bass.Bass):
    qtok = nc.dram_tensor("qtok", [OWN], I32, kind="ExternalInput")
    E = nc.dram_tensor("E", [VOCAB, D], BF16, kind="ExternalInput")
    WqT = nc.dram_tensor("WqT", [D, D], BF16, kind="ExternalInput")
    WkT = nc.dram_tensor("WkT", [D, D], BF16, kind="ExternalInput")
    WvT = nc.dram_tensor("WvT", [D, D], BF16, kind="ExternalInput")
    bq = nc.dram_tensor("bq", [D], F32, kind="ExternalInput")
    bk = nc.dram_tensor("bk", [D], F32, kind="ExternalInput")
    Wp8 = nc.dram_tensor("Wp8", [D, VSH], FP8, kind="ExternalInput")
    # ridx_sh[r, jb] = global_row(r) - jb*512, fp32
    ridx_sh = nc.dram_tensor("ridx_sh", [OWN, JB], F32, kind="ExternalInput")
    logits = nc.dram_tensor("logits", [N_TOK, VSH], BF16, kind="ExternalOutput")

    with tile.TileContext(nc) as tc:
        with (
            tc.tile_pool(name="const", bufs=1) as const,
            tc.tile_pool(name="dram", bufs=1, space="DRAM") as dram,
        ):
            ident = const.tile([P, P], BF16)
            make_identity(nc, ident[:])

            bq_t = const.tile([P, OT], F32)
            nc.sync.dma_start(bq_t[:], bq.ap().rearrange("(ot p) -> p ot", p=P))
            bk_t = const.tile([P, OT], F32)
            nc.sync.dma_start(bk_t[:], bk.ap().rearrange("(ot p) -> p ot", p=P))

            rsh = const.tile([P, IBLK, JB], F32)
            nc.sync.dma_start(
                rsh[:], ridx_sh.ap().rearrange("(ib p) jb -> p ib jb", p=P)
            )

            jidx0 = const.tile([P, 512], F32)

            qtok_sb = const.tile([P, OWN // P], I32)
            nc.sync.dma_start(qtok_sb[:], qtok.ap().rearrange("(g p) -> p g", p=P))

            # DRAM scratch for collectives
            warm_in = dram.tile([P], BF16)
            warm_out = dram.tile([NC, P], BF16, addr_space="Shared")
            kv_send = dram.tile([KV_ELEMS], FP8)
            kvg = dram.tile([NC, KV_ELEMS], FP8, addr_space="Shared")
            oTb = [dram.tile([P, KT, P], FP8, name=f"oTb{q}") for q in range(IBLK)]
            gat = [
                dram.tile(
                    [NC, P, KT, P], FP8, name=f"gat{q}", addr_space="Shared"
                )
                for q in range(IBLK)
            ]

            # ---------------- phase QKV: own tokens only --------------------
            qT_pool = tc.alloc_tile_pool(name="qT_keep", bufs=1)
            qT = qT_pool.tile([P, OT, OWN], BF16)
            kT_pool = tc.alloc_tile_pool(name="kT_keep", bufs=1)
            kT_all = kT_pool.tile([P, OT, N_TOK], FP8)
            with (
                tc.tile_pool(name="sbw", bufs=1) as sbw,
                tc.tile_pool(name="sbq", bufs=2) as sbq,
                tc.tile_pool(name="psq_tr", bufs=2, space="PSUM") as psq_tr,
                tc.tile_pool(name="psq_pp", bufs=2, space="PSUM") as psq_pp,
                tc.tile_pool(name="psq_pv", bufs=2, space="PSUM") as psq_pv,
            ):
                wz = sbq.tile([1, P], BF16, tag="wz")
                nc.vector.memset(wz[:], 0.0)
                nc.sync.dma_start(warm_in[:][None, :], wz[:])
                nc.gpsimd.collective_compute(
                    "AllGather",
                    mybir.AluOpType.bypass,
                    replica_groups=[list(range(NC))],
                    ins=[warm_in.opt()],
                    outs=[warm_out.opt()],
                )

                ji = sbw.tile([P, 512], I32, tag="ji")
                nc.gpsimd.iota(ji[:], pattern=[[1, 512]], base=0, channel_multiplier=0)
                nc.vector.tensor_copy(out=jidx0[:], in_=ji[:])

                wq_b = sbw.tile([P, KT, D], BF16, tag="wq")
                nc.sync.dma_start(
                    wq_b[:], WqT.ap().rearrange("(kt p) o -> p kt o", p=P)
                )
                wk_b = sbw.tile([P, KT, D], BF16, tag="wk")
                nc.sync.dma_start(
                    wk_b[:], WkT.ap().rearrange("(kt p) o -> p kt o", p=P)
                )
                wv_b = sbw.tile([P, KT, D], BF16, tag="wv")
                nc.sync.dma_start(
                    wv_b[:], WvT.ap().rearrange("(kt p) o -> p kt o", p=P)
                )

                # gather own embeddings + transpose -> xT [P, KT, OWN]
                xT = sbw.tile([P, KT, OWN], BF16, tag="xT")
                for g in range(IBLK):
                    xg = sbq.tile([P, D], BF16, tag="xg")
                    nc.gpsimd.indirect_dma_start(
                        out=xg[:],
                        out_offset=None,
                        in_=E.ap(),
                        in_offset=bass.IndirectOffsetOnAxis(
                            ap=qtok_sb[:, g : g + 1], axis=0
                        ),
                    )
                    for kt in range(KT):
                        pst = psq_tr.tile([P, P], BF16, tag="ptr")
                        nc.tensor.transpose(
                            pst[:], xg[:, kt * P : (kt + 1) * P], ident[:]
                        )
                        nc.vector.tensor_copy(
                            out=xT[:, kt, g * P : (g + 1) * P], in_=pst[:]
                        )

                # K^T for own tokens -> kv_send[0:KV_K]
                kT_own = sbw.tile([P, OT, OWN], FP8, tag="kT_own")
                for ot in range(OT):
                    pk = psq_pp.tile([P, OWN], F32, tag="pp")
                    for kt in range(KT):
                        nc.tensor.matmul(
                            pk[:],
                            lhsT=wk_b[:, kt, ot * P : (ot + 1) * P],
                            rhs=xT[:, kt, :],
                            start=(kt == 0),
                            stop=(kt == KT - 1),
                        )
                    nc.vector.tensor_scalar(
                        out=kT_own[:, ot, :],
                        in0=pk[:],
                        scalar1=bk_t[:, ot : ot + 1],
                        scalar2=16.0,
                        op0=mybir.AluOpType.add,
                        op1=mybir.AluOpType.mult,
                    )
                nc.sync.dma_start(
                    kv_send[0:KV_K].rearrange(
                        "(p ot j) -> p ot j", p=P, ot=OT
                    ),
                    kT_own[:],
                )

                # V for own tokens (NO bias - folded into host bias_row)
                for tb in range(IBLK):
                    pv = psq_pv.tile([P, D], F32, tag="pv")
                    for half in range(2):
                        for kt in range(KT):
                            nc.tensor.matmul(
                                pv[:, half * 512 : (half + 1) * 512],
                                lhsT=xT[:, kt, tb * P : (tb + 1) * P],
                                rhs=wv_b[:, kt, half * 512 : (half + 1) * 512],
                                start=(kt == 0),
                                stop=(kt == KT - 1),
                            )
                    ve = sbq.tile([P, D], FP8, tag="ve")
                    nc.vector.tensor_scalar(
                        out=ve[:],
                        in0=pv[:],
                        scalar1=16.0,
                        scalar2=None,
                        op0=mybir.AluOpType.mult,
                    )
                    nc.sync.dma_start(
                        kv_send[
                            KV_K + tb * KV_V : KV_K + (tb + 1) * KV_V
                        ].rearrange("(p d) -> p d", p=P),
                        ve[:],
                    )

                nc.gpsimd.collective_compute(
                    "AllGather",
                    mybir.AluOpType.bypass,
                    replica_groups=[list(range(NC))],
                    ins=[kv_send.opt()],
                    outs=[kvg.opt()],
                )

                # Q^T (scaled by 1/sqrt(d)) kept in SBUF
                for ot in range(OT):
                    pp = psq_pp.tile([P, OWN], F32, tag="pp")
                    for kt in range(KT):
                        nc.tensor.matmul(
                            pp[:],
                            lhsT=wq_b[:, kt, ot * P : (ot + 1) * P],
                            rhs=xT[:, kt, :],
                            start=(kt == 0),
                            stop=(kt == KT - 1),
                        )
                    nc.vector.tensor_scalar(
                        out=qT[:, ot, :],
                        in0=pp[:],
                        scalar1=bq_t[:, ot : ot + 1],
                        scalar2=SCALE / 16.0,
                        op0=mybir.AluOpType.add,
                        op1=mybir.AluOpType.mult,
                    )


            # ---------------- load gathered K^T + Wp + bias -----------------
            wp_pool = tc.alloc_tile_pool(name="wp_keep", bufs=1)
            wp_b = wp_pool.tile([P, KT, VSH], FP8)
            nc.sync.dma_start(
                wp_b[:], Wp8.ap().rearrange("(kt p) v -> p kt v", p=P)
            )
            for r in range(NC):
                nc.sync.dma_start(
                    kT_all[:, :, r * 512 : (r + 1) * 512],
                    kvg[r, 0:KV_K].rearrange("(p ot j) -> p ot j", p=P, ot=OT),
                )

            # ---------------- phase attention (own rows) --------------------
            with (
                tc.tile_pool(name="sbat", bufs=2) as sbat,
                tc.tile_pool(name="psat_sc", bufs=2, space="PSUM") as ps_sc,
                tc.tile_pool(name="psat_av", bufs=1, space="PSUM") as ps_av,
                tc.tile_pool(name="psat_tr", bufs=2, space="PSUM") as ps_tr,
            ):
                for ib in range(IBLK):
                    a_row = sbat.tile([P, N_TOK], BF16, tag="a_row")
                    for jb in range(JB):
                        ps = ps_sc.tile([P, 512], F32, tag="sc")
                        for ot in range(OT):
                            nc.tensor.matmul(
                                ps[:],
                                lhsT=qT[:, ot, ib * P : (ib + 1) * P],
                                rhs=kT_all[:, ot, jb * 512 : (jb + 1) * 512],
                                start=(ot == 0),
                                stop=(ot == OT - 1),
                            )
                        astr = a_row[:, jb * 512 : (jb + 1) * 512]
                        nc.scalar.activation(
                            astr, ps[:], mybir.ActivationFunctionType.Exp
                        )
                        # multiply by causal mask: (jidx0 <= ridx - jb*512) * exp
                        nc.vector.scalar_tensor_tensor(
                            out=astr,
                            in0=jidx0[:],
                            scalar=rsh[:, ib, jb : jb + 1],
                            in1=astr,
                            op0=mybir.AluOpType.is_le,
                            op1=mybir.AluOpType.mult,
                        )
                    dsum = sbat.tile([P, 1], F32, tag="dsum")
                    nc.vector.tensor_reduce(
                        out=dsum[:],
                        in_=a_row[:],
                        axis=mybir.AxisListType.X,
                        op=mybir.AluOpType.add,
                    )
                    rden = sbat.tile([P, 1], F32, tag="rden")
                    nc.vector.reciprocal(rden[:], dsum[:])

                    pav = ps_av.tile([P, D], F32, tag="av")
                    for j2 in range(JB2):
                        pat = ps_tr.tile([P, P], BF16, tag="tr")
                        nc.tensor.transpose(
                            pat[:], a_row[:, j2 * P : (j2 + 1) * P], ident[:]
                        )
                        at = sbat.tile([P, P], BF16, tag="at")
                        nc.vector.tensor_copy(out=at[:], in_=pat[:])
                        vj = sbat.tile([P, D], FP8, tag="vj")
                        r, tb = j2 // IBLK, j2 % IBLK
                        nc.sync.dma_start(
                            vj[:],
                            kvg[
                                r, KV_K + tb * KV_V : KV_K + (tb + 1) * KV_V
                            ].rearrange("(p d) -> p d", p=P),
                        )
                        nc.tensor.matmul(
                            pav[:, 0:512],
                            lhsT=at[:],
                            rhs=vj[:, 0:512],
                            start=(j2 == 0),
                            stop=(j2 == JB2 - 1),
                        )
                        nc.tensor.matmul(
                            pav[:, 512:1024],
                            lhsT=at[:],
                            rhs=vj[:, 512:1024],
                            start=(j2 == 0),
                            stop=(j2 == JB2 - 1),
                        )
                    o_bf = sbat.tile([P, D], BF16, tag="o_bf")
                    nc.vector.tensor_scalar(
                        out=o_bf[:],
                        in0=pav[:],
                        scalar1=rden[:, :1],
                        scalar2=1.0 / 16.0,
                        op0=mybir.AluOpType.mult,
                        op1=mybir.AluOpType.mult,
                    )
                    oT = sbat.tile([P, KT, P], FP8, tag="oT")
                    for kt in range(KT):
                        pot = ps_tr.tile([P, P], BF16, tag="tr")
                        nc.tensor.transpose(
                            pot[:], o_bf[:, kt * P : (kt + 1) * P], ident[:]
                        )
                        nc.vector.tensor_scalar(
                            out=oT[:, kt, :],
                            in0=pot[:],
                            scalar1=OUT_SCALE,
                            scalar2=None,
                            op0=mybir.AluOpType.mult,
                        )
                    nc.sync.dma_start(oTb[ib][:], oT[:])
                    nc.gpsimd.collective_compute(
                        "AllGather",
                        mybir.AluOpType.bypass,
                        replica_groups=[list(range(NC))],
                        ins=[oTb[ib].opt()],
                        outs=[gat[ib].opt()],
                    )

            # ---------------- phase logits ----------------------------------
            with (
                tc.tile_pool(name="sblg", bufs=2) as sblg,
                tc.tile_pool(name="pslg", bufs=7, space="PSUM") as pslg,
            ):
                for q in range(IBLK):
                    for c in range(NC):
                        ibg = c * IBLK + q  # global row-block
                        lt8 = sblg.tile([P, KT, P], FP8, tag="lt8")
                        nc.sync.dma_start(lt8[:], gat[q][c, :, :, :])
                        for si, (v0, vw) in enumerate(VSTRIPS):
                            pl = pslg.tile([P, 512], F32, tag="lg")
                            for k2 in range(KT // 2):
                                nc.tensor.matmul(
                                    pl[:, :vw],
                                    lhsT=lt8[:, 2 * k2 : 2 * k2 + 2, :],
                                    rhs=wp_b[:, 2 * k2 : 2 * k2 + 2, v0 : v0 + vw],
                                    start=(k2 == 0),
                                    stop=(k2 == KT // 2 - 1),
                                    perf_mode=mybir.MatmulPerfMode.DoubleRow,
                                )
                            lo = sblg.tile([P, 512], BF16, tag="lo")
                            if si % 2 == 0:
                                nc.scalar.activation(
                                    lo[:, :vw],
                                    pl[:, :vw],
                                    mybir.ActivationFunctionType.Copy,
                                )
                            else:
                                nc.vector.tensor_copy(
                                    out=lo[:, :vw], in_=pl[:, :vw]
                                )
                            nc.sync.dma_start(
                                logits.ap()[
                                    ibg * P : (ibg + 1) * P, v0 : v0 + vw
                                ],
                                lo[:, :vw],
                            )
            wp_pool.release()
            kT_pool.release()
            qT_pool.release()
    return nc


def _prep_inputs(inputs):
    """Host-side shard prep: slicing, transposes, padding, dtype pre-casts."""
    tokens = np.ascontiguousarray(np.asarray(inputs["tokens"]).astype(np.int32))
    E16 = np.asarray(inputs["E"], np.float32).astype(ml_dtypes.bfloat16)
    WqT = np.ascontiguousarray(
        np.asarray(inputs["Wq"], np.float32).T.astype(ml_dtypes.bfloat16)
    )
    WkT = np.ascontiguousarray(
        np.asarray(inputs["Wk"], np.float32).T.astype(ml_dtypes.bfloat16)
    )
    WvT = np.ascontiguousarray(
        np.asarray(inputs["Wv"], np.float32).T.astype(ml_dtypes.bfloat16)
    )
    Wp = np.asarray(inputs["Wp"], np.float32)
    bv = np.asarray(inputs["bv"], np.float32)
    WpT_pad = np.zeros((D, VPAD_TOT), np.float32)
    WpT_pad[:, :VOCAB] = Wp.T
    Wp8_full = (WpT_pad * WP_SCALE).astype(ml_dtypes.float8_e4m3)
    bias_full = np.zeros((VPAD_TOT,), np.float32)
    bias_full[:VOCAB] = np.asarray(inputs["bp"], np.float32) + Wp @ bv

    in_maps = []
    for c in range(NC):
        rows = np.arange(c * OWN, (c + 1) * OWN, dtype=np.float32)
        ridx_sh = rows[:, None] - 512.0 * np.arange(JB, dtype=np.float32)[None, :]
        in_maps.append(
            {
                "qtok": np.ascontiguousarray(tokens[c * OWN : (c + 1) * OWN]),
                "E": E16,
                "WqT": WqT,
                "WkT": WkT,
                "WvT": WvT,
                "bq": np.asarray(inputs["bq"], np.float32),
                "bk": np.asarray(inputs["bk"], np.float32),
                "Wp8": np.ascontiguousarray(Wp8_full[:, c * VSH : (c + 1) * VSH]),
                "ridx_sh": np.ascontiguousarray(ridx_sh, dtype=np.float32),
            }
        )
    return in_maps, bias_full


def _run(inputs, trace=False):
    nc = bacc.Bacc(trn_type="TRN2", num_devices=NC, debug=False)
    build(nc)
    nc.compile()
    in_maps, bias_full = _prep_inputs(inputs)
    res = run_bass_kernel_spmd(
        nc, in_maps, core_ids=list(range(NC)), trace=trace
    )
    dq = 1.0 / (WP_SCALE * OUT_SCALE)
    out = np.concatenate(
        [
            np.asarray(res.results[c]["logits"], np.float32) * dq
            + bias_full[None, c * VSH : (c + 1) * VSH]
            for c in range(NC)
        ],
        axis=1,
    )[:, :VOCAB]
    return out, res


def kernel(**inputs) -> np.ndarray:
    out, _ = _run(inputs, trace=False)
    return out


# revision 7
# speedup vs baseline: 1.7887x; 1.2334x over previous
"""Self-contained Trainium2 Bass kernel for nn_CharModel (dense transformer
forward: embed -> single-head causal attention -> vocab projection).

Distribution over 8 NeuronCores:
  - sequence-parallel QKV: core c computes Q/K/V only for its own 512 tokens,
    K^T and V are exchanged with one bf16 AllGather (2MB/rank)
  - sequence-parallel attention rows, vocab-parallel logits (6400 cols/core)
  - attention outputs exchanged with 4 chunked bf16 AllGathers
Host pre-casts: E/Wq/Wk/Wv to bf16, Wp.T*64 to fp8e4, and folds bv@Wp.T+bp
into a single f32 bias row. Logits are written bf16 (host upcasts).
"""
import numpy as np
import ml_dtypes

import concourse.bass as bass
import concourse.mybir as mybir
import concourse.tile as tile
from concourse import bacc
from concourse.bass_utils import run_bass_kernel_spmd
from concourse.masks import make_identity

P = 128
N_TOK = 4096
D = 1024
VOCAB = 50257
NC = 8
VPAD_TOT = 51200  # 50257 padded up to 400*128
VSH = VPAD_TOT // NC  # 6400 per-core vocab shard
OWN = N_TOK // NC  # 512 own tokens
IBLK = OWN // P  # 4 own row-blocks
KT = D // P  # 8 contraction tiles
OT = D // P  # 8 output-feature tiles
JB = N_TOK // 512  # 8 key strips of 512
JB2 = N_TOK // P  # 32 key tiles of 128
SCALE = 1.0 / 32.0  # 1/sqrt(D)

F32 = mybir.dt.float32
BF16 = mybir.dt.bfloat16
FP8 = mybir.dt.float8e4
I32 = mybir.dt.int32
WP_SCALE = 64.0
OUT_SCALE = 256.0

KV_K = P * OT * 512  # kT section elements in the kv exchange buffer
KV_V = P * D  # one V tile (128 tokens x 1024)
KV_ELEMS = KV_K + IBLK * KV_V

# logits v-strips within the 6400-wide shard: 12 x 512 + 1 x 256
VSTRIPS = [(i * 512, 512) for i in range(12)] + [(6144, 256)]


def build(nc: bass.Bass):
    qtok = nc.dram_tensor("qtok", [OWN], I32, kind="ExternalInput")
    E = nc.dram_tensor("E", [VOCAB, D], BF16, kind="ExternalInput")
    WqT = nc.dram_tensor("WqT", [D, D], BF16, kind="ExternalInput")
    WkT = nc.dram_tensor("WkT", [D, D], BF16, kind="ExternalInput")
    WvT = nc.dram_tensor("WvT", [D, D], BF16, kind="ExternalInput")
    bq = nc.dram_tensor("bq", [D], F32, kind="ExternalInput")
    bk = nc.dram_tensor("bk", [D], F32, kind="ExternalInput")
    Wp8 = nc.dram_tensor("Wp8", [D, VSH], FP8, kind="ExternalInput")
    # ridx_sh[r, jb] = global_row(r) - jb*512, fp32
    ridx_sh = nc.dram_tensor("ridx_sh", [OWN, JB], F32, kind="ExternalInput")
    logits = nc.dram_tensor("logits", [N_TOK, VSH], BF16, kind="ExternalOutput")

    with tile.TileContext(nc) as tc:
        with (
            tc.tile_pool(name="const", bufs=1) as const,
            tc.tile_pool(name="dram", bufs=1, space="DRAM") as dram,
        ):
            ident = const.tile([P, P], BF16)
            make_identity(nc, ident[:])

            bq_t = const.tile([P, OT], F32)
            nc.sync.dma_start(bq_t[:], bq.ap().rearrange("(ot p) -> p ot", p=P))
            bk_t = const.tile([P, OT], F32)
            nc.sync.dma_start(bk_t[:], bk.ap().rearrange("(ot p) -> p ot", p=P))

            rsh = const.tile([P, IBLK, JB], F32)
            nc.sync.dma_start(
                rsh[:], ridx_sh.ap().rearrange("(ib p) jb -> p ib jb", p=P)
            )

            jidx0 = const.tile([P, 512], F32)

            qtok_sb = const.tile([P, OWN // P], I32)
            nc.sync.dma_start(qtok_sb[:], qtok.ap().rearrange("(g p) -> p g", p=P))

            # DRAM scratch for collectives
            warm_in = dram.tile([P], BF16)
            warm_out = dram.tile([NC, P], BF16, addr_space="Shared")
            kv_send = dram.tile([KV_ELEMS], FP8)
            kvg = dram.tile([NC, KV_ELEMS], FP8, addr_space="Shared")
            oTb = [dram.tile([P, KT, P], FP8, name=f"oTb{q}") for q in range(IBLK)]
            gat = [
                dram.tile(
                    [NC, P, KT, P], FP8, name=f"gat{q}", addr_space="Shared"
                )
                for q in range(IBLK)
            ]

            # ---------------- phase QKV: own tokens only --------------------
            qT_pool = tc.alloc_tile_pool(name="qT_keep", bufs=1)
            qT = qT_pool.tile([P, OT, OWN], BF16)
            kT_pool = tc.alloc_tile_pool(name="kT_keep", bufs=1)
            kT_all = kT_pool.tile([P, OT, N_TOK], FP8)
            with (
                tc.tile_pool(name="sbw", bufs=1) as sbw,
                tc.tile_pool(name="sbq", bufs=2) as sbq,
                tc.tile_pool(name="psq_tr", bufs=2, space="PSUM") as psq_tr,
                tc.tile_pool(name="psq_pp", bufs=2, space="PSUM") as psq_pp,
                tc.tile_pool(name="psq_pv", bufs=2, space="PSUM") as psq_pv,
            ):
                wz = sbq.tile([1, P], BF16, tag="wz")
                nc.vector.memset(wz[:], 0.0)
                nc.sync.dma_start(warm_in[:][None, :], wz[:])
                nc.gpsimd.collective_compute(
                    "AllGather",
                    mybir.AluOpType.bypass,
                    replica_groups=[list(range(NC))],
                    ins=[warm_in.opt()],
                    outs=[warm_out.opt()],
                )

                ji = sbw.tile([P, 512], I32, tag="ji")
                nc.gpsimd.iota(ji[:], pattern=[[1, 512]], base=0, channel_multiplier=0)
                nc.vector.tensor_copy(out=jidx0[:], in_=ji[:])

                wq_b = sbw.tile([P, KT, D], BF16, tag="wq")
                nc.sync.dma_start(
                    wq_b[:], WqT.ap().rearrange("(kt p) o -> p kt o", p=P)
                )
                wk_b = sbw.tile([P, KT, D], BF16, tag="wk")
                nc.sync.dma_start(
                    wk_b[:], WkT.ap().rearrange("(kt p) o -> p kt o", p=P)
                )
                wv_b = sbw.tile([P, KT, D], BF16, tag="wv")
                nc.sync.dma_start(
                    wv_b[:], WvT.ap().rearrange("(kt p) o -> p kt o", p=P)
                )

                # gather own embeddings + transpose -> xT [P, KT, OWN]
                xT = sbw.tile([P, KT, OWN], BF16, tag="xT")
                for g in range(IBLK):
                    xg = sbq.tile([P, D], BF16, tag="xg")
                    nc.gpsimd.indirect_dma_start(
                        out=xg[:],
                        out_offset=None,
                        in_=E.ap(),
                        in_offset=bass.IndirectOffsetOnAxis(
                            ap=qtok_sb[:, g : g + 1], axis=0
                        ),
                    )
                    for kt in range(KT):
                        pst = psq_tr.tile([P, P], BF16, tag="ptr")
                        nc.tensor.transpose(
                            pst[:], xg[:, kt * P : (kt + 1) * P], ident[:]
                        )
                        nc.vector.tensor_copy(
                            out=xT[:, kt, g * P : (g + 1) * P], in_=pst[:]
                        )

                # K^T for own tokens -> kv_send[0:KV_K]
                kT_own = sbw.tile([P, OT, OWN], FP8, tag="kT_own")
                for ot in range(OT):
                    pk = psq_pp.tile([P, OWN], F32, tag="pp")
                    for kt in range(KT):
                        nc.tensor.matmul(
                            pk[:],
                            lhsT=wk_b[:, kt, ot * P : (ot + 1) * P],
                            rhs=xT[:, kt, :],
                            start=(kt == 0),
                            stop=(kt == KT - 1),
                        )
                    nc.vector.tensor_scalar(
                        out=kT_own[:, ot, :],
                        in0=pk[:],
                        scalar1=bk_t[:, ot : ot + 1],
                        scalar2=16.0,
                        op0=mybir.AluOpType.add,
                        op1=mybir.AluOpType.mult,
                    )
                nc.sync.dma_start(
                    kv_send[0:KV_K].rearrange(
                        "(p ot j) -> p ot j", p=P, ot=OT
                    ),
                    kT_own[:],
                )

                # V for own tokens (NO bias - folded into host bias_row)
                for tb in range(IBLK):
                    pv = psq_pv.tile([P, D], F32, tag="pv")
                    for half in range(2):
                        for kt in range(KT):
                            nc.tensor.matmul(
                                pv[:, half * 512 : (half + 1) * 512],
                                lhsT=xT[:, kt, tb * P : (tb + 1) * P],
                                rhs=wv_b[:, kt, half * 512 : (half + 1) * 512],
                                start=(kt == 0),
                                stop=(kt == KT - 1),
                            )
                    ve = sbq.tile([P, D], FP8, tag="ve")
                    nc.vector.tensor_scalar(
                        out=ve[:],
                        in0=pv[:],
                        scalar1=16.0,
                        scalar2=None,
                        op0=mybir.AluOpType.mult,
                    )
                    nc.sync.dma_start(
                        kv_send[
                            KV_K + tb * KV_V : KV_K + (tb + 1) * KV_V
                        ].rearrange("(p d) -> p d", p=P),
                        ve[:],
                    )

                nc.gpsimd.collective_compute(
                    "AllGather",
                    mybir.AluOpType.bypass,
                    replica_groups=[list(range(NC))],
                    ins=[kv_send.opt()],
                    outs=[kvg.opt()],
                )

                # Q^T (scaled by 1/sqrt(d)) kept in SBUF
                for ot in range(OT):
                    pp = psq_pp.tile([P, OWN], F32, tag="pp")
                    for kt in range(KT):
                        nc.tensor.matmul(
                            pp[:],
                            lhsT=wq_b[:, kt, ot * P : (ot + 1) * P],
                            rhs=xT[:, kt, :],
                            start=(kt == 0),
                            stop=(kt == KT - 1),
                        )
                    nc.vector.tensor_scalar(
                        out=qT[:, ot, :],
                        in0=pp[:],
                        scalar1=bq_t[:, ot : ot + 1],
                        scalar2=SCALE / 16.0,
                        op0=mybir.AluOpType.add,
                        op1=mybir.AluOpType.mult,
                    )


            # ---------------- load gathered K^T + Wp + bias -----------------
            wp_pool = tc.alloc_tile_pool(name="wp_keep", bufs=1)
            wp_b = wp_pool.tile([P, KT, VSH], FP8)
            nc.sync.dma_start(
                wp_b[:], Wp8.ap().rearrange("(kt p) v -> p kt v", p=P)
            )
            for r in range(NC):
                nc.sync.dma_start(
                    kT_all[:, :, r * 512 : (r + 1) * 512],
                    kvg[r, 0:KV_K].rearrange("(p ot j) -> p ot j", p=P, ot=OT),
                )

            # ---------------- phase attention (own rows) --------------------
            with (
                tc.tile_pool(name="sbat", bufs=2) as sbat,
                tc.tile_pool(name="psat_sc", bufs=2, space="PSUM") as ps_sc,
                tc.tile_pool(name="psat_av", bufs=1, space="PSUM") as ps_av,
                tc.tile_pool(name="psat_tr", bufs=2, space="PSUM") as ps_tr,
            ):
                for ib in range(IBLK):
                    a_row = sbat.tile([P, N_TOK], BF16, tag="a_row")
                    for jb in range(JB):
                        ps = ps_sc.tile([P, 512], F32, tag="sc")
                        for ot in range(OT):
                            nc.tensor.matmul(
                                ps[:],
                                lhsT=qT[:, ot, ib * P : (ib + 1) * P],
                                rhs=kT_all[:, ot, jb * 512 : (jb + 1) * 512],
                                start=(ot == 0),
                                stop=(ot == OT - 1),
                            )
                        astr = a_row[:, jb * 512 : (jb + 1) * 512]
                        nc.scalar.activation(
                            astr, ps[:], mybir.ActivationFunctionType.Exp
                        )
                        # multiply by causal mask: (jidx0 <= ridx - jb*512) * exp
                        nc.vector.scalar_tensor_tensor(
                            out=astr,
                            in0=jidx0[:],
                            scalar=rsh[:, ib, jb : jb + 1],
                            in1=astr,
                            op0=mybir.AluOpType.is_le,
                            op1=mybir.AluOpType.mult,
                        )
                    dsum = sbat.tile([P, 1], F32, tag="dsum")
                    nc.vector.tensor_reduce(
                        out=dsum[:],
                        in_=a_row[:],
                        axis=mybir.AxisListType.X,
                        op=mybir.AluOpType.add,
                    )
                    rden = sbat.tile([P, 1], F32, tag="rden")
                    nc.vector.reciprocal(rden[:], dsum[:])

                    pav = ps_av.tile([P, D], F32, tag="av")
                    for j2 in range(JB2):
                        pat = ps_tr.tile([P, P], BF16, tag="tr")
                        nc.tensor.transpose(
                            pat[:], a_row[:, j2 * P : (j2 + 1) * P], ident[:]
                        )
                        at = sbat.tile([P, P], BF16, tag="at")
                        nc.vector.tensor_copy(out=at[:], in_=pat[:])
                        vj = sbat.tile([P, D], FP8, tag="vj")
                        r, tb = j2 // IBLK, j2 % IBLK
                        nc.sync.dma_start(
                            vj[:],
                            kvg[
                                r, KV_K + tb * KV_V : KV_K + (tb + 1) * KV_V
                            ].rearrange("(p d) -> p d", p=P),
                        )
                        nc.tensor.matmul(
                            pav[:, 0:512],
                            lhsT=at[:],
                            rhs=vj[:, 0:512],
                            start=(j2 == 0),
                            stop=(j2 == JB2 - 1),
                        )
                        nc.tensor.matmul(
                            pav[:, 512:1024],
                            lhsT=at[:],
                            rhs=vj[:, 512:1024],
                            start=(j2 == 0),
                            stop=(j2 == JB2 - 1),
                        )
                    o_bf = sbat.tile([P, D], BF16, tag="o_bf")
                    nc.vector.tensor_scalar(
                        out=o_bf[:],
                        in0=pav[:],
                        scalar1=rden[:, :1],
                        scalar2=1.0 / 16.0,
                        op0=mybir.AluOpType.mult,
                        op1=mybir.AluOpType.mult,
                    )
                    oT = sbat.tile([P, KT, P], FP8, tag="oT")
                    for kt in range(KT):
                        pot = ps_tr.tile([P, P], BF16, tag="tr")
                        nc.tensor.transpose(
                            pot[:], o_bf[:, kt * P : (kt + 1) * P], ident[:]
                        )
                        nc.vector.tensor_scalar(
                            out=oT[:, kt, :],
                            in0=pot[:],
                            scalar1=OUT_SCALE,
                            scalar2=None,
                            op0=mybir.AluOpType.mult,
                        )
                    nc.sync.dma_start(oTb[ib][:], oT[:])
                    nc.gpsimd.collective_compute(
                        "AllGather",
                        mybir.AluOpType.bypass,
                        replica_groups=[list(range(NC))],
                        ins=[oTb[ib].opt()],
                        outs=[gat[ib].opt()],
                    )

            # ---------------- phase logits ----------------------------------
            with (
                tc.tile_pool(name="sblg", bufs=3) as sblg,
                tc.tile_pool(name="sblo", bufs=8) as sblo,
                tc.tile_pool(name="pslg", bufs=7, space="PSUM") as pslg,
            ):
                for q in range(IBLK):
                    for c in range(NC):
                        ibg = c * IBLK + q  # global row-block
                        lt8 = sblg.tile([P, KT, P], FP8, tag="lt8")
                        nc.sync.dma_start(lt8[:], gat[q][c, :, :, :])
                        for si, (v0, vw) in enumerate(VSTRIPS):
                            pl = pslg.tile([P, 512], F32, tag="lg")
                            for k2 in range(KT // 2):
                                nc.tensor.matmul(
                                    pl[:, :vw],
                                    lhsT=lt8[:, 2 * k2 : 2 * k2 + 2, :],
                                    rhs=wp_b[:, 2 * k2 : 2 * k2 + 2, v0 : v0 + vw],
                                    start=(k2 == 0),
                                    stop=(k2 == KT // 2 - 1),
                                    perf_mode=mybir.MatmulPerfMode.DoubleRow,
                                )
                            lo = sblo.tile([P, 512], BF16, tag="lo")
                            h = vw // 2
                            nc.scalar.activation(
                                lo[:, :h],
                                pl[:, :h],
                                mybir.ActivationFunctionType.Copy,
                            )
                            nc.vector.tensor_copy(
                                out=lo[:, h:vw], in_=pl[:, h:vw]
                            )
                            nc.gpsimd.dma_start(
                                logits.ap()[
                                    ibg * P : (ibg + 1) * P, v0 : v0 + vw
                                ],
                                lo[:, :vw],
                            )
            wp_pool.release()
            kT_pool.release()
            qT_pool.release()
    return nc


def _prep_inputs(inputs):
    """Host-side shard prep: slicing, transposes, padding, dtype pre-casts."""
    tokens = np.ascontiguousarray(np.asarray(inputs["tokens"]).astype(np.int32))
    E16 = np.asarray(inputs["E"], np.float32).astype(ml_dtypes.bfloat16)
    WqT = np.ascontiguousarray(
        np.asarray(inputs["Wq"], np.float32).T.astype(ml_dtypes.bfloat16)
    )
    WkT = np.ascontiguousarray(
        np.asarray(inputs["Wk"], np.float32).T.astype(ml_dtypes.bfloat16)
    )
    WvT = np.ascontiguousarray(
        np.asarray(inputs["Wv"], np.float32).T.astype(ml_dtypes.bfloat16)
    )
    Wp = np.asarray(inputs["Wp"], np.float32)
    bv = np.asarray(inputs["bv"], np.float32)
    WpT_pad = np.zeros((D, VPAD_TOT), np.float32)
    WpT_pad[:, :VOCAB] = Wp.T
    Wp8_full = (WpT_pad * WP_SCALE).astype(ml_dtypes.float8_e4m3)
    bias_full = np.zeros((VPAD_TOT,), np.float32)
    bias_full[:VOCAB] = np.asarray(inputs["bp"], np.float32) + Wp @ bv

    in_maps = []
    for c in range(NC):
        rows = np.arange(c * OWN, (c + 1) * OWN, dtype=np.float32)
        ridx_sh = rows[:, None] - 512.0 * np.arange(JB, dtype=np.float32)[None, :]
        in_maps.append(
            {
                "qtok": np.ascontiguousarray(tokens[c * OWN : (c + 1) * OWN]),
                "E": E16,
                "WqT": WqT,
                "WkT": WkT,
                "WvT": WvT,
                "bq": np.asarray(inputs["bq"], np.float32),
                "bk": np.asarray(inputs["bk"], np.float32),
                "Wp8": np.ascontiguousarray(Wp8_full[:, c * VSH : (c + 1) * VSH]),
                "ridx_sh": np.ascontiguousarray(ridx_sh, dtype=np.float32),
            }
        )
    return in_maps, bias_full


def _run(inputs, trace=False):
    nc = bacc.Bacc(trn_type="TRN2", num_devices=NC, debug=False)
    build(nc)
    nc.compile()
    in_maps, bias_full = _prep_inputs(inputs)
    res = run_bass_kernel_spmd(
        nc, in_maps, core_ids=list(range(NC)), trace=trace
    )
    dq = 1.0 / (WP_SCALE * OUT_SCALE)
    out = np.concatenate(
        [
            np.asarray(res.results[c]["logits"], np.float32) * dq
            + bias_full[None, c * VSH : (c + 1) * VSH]
            for c in range(NC)
        ],
        axis=1,
    )[:, :VOCAB]
    return out, res


def kernel(**inputs) -> np.ndarray:
    out, _ = _run(inputs, trace=False)
    return out


# revision 9
# speedup vs baseline: 1.9525x; 1.0916x over previous
"""Self-contained Trainium2 Bass kernel for nn_CharModel (dense transformer
forward: embed -> single-head causal attention -> vocab projection).

Distribution over 8 NeuronCores:
  - sequence-parallel QKV: core c computes Q/K/V only for its own 512 tokens,
    K^T and V are exchanged with one bf16 AllGather (2MB/rank)
  - sequence-parallel attention rows, vocab-parallel logits (6400 cols/core)
  - attention outputs exchanged with 4 chunked bf16 AllGathers
Host pre-casts: E/Wq/Wk/Wv to bf16, Wp.T*64 to fp8e4, and folds bv@Wp.T+bp
into a single f32 bias row. Logits are written bf16 (host upcasts).
"""
import numpy as np
import ml_dtypes

import concourse.bass as bass
import concourse.mybir as mybir
import concourse.tile as tile
from concourse import bacc
from concourse.bass_utils import run_bass_kernel_spmd
from concourse.masks import make_identity

P = 128
N_TOK = 4096
D = 1024
VOCAB = 50257
NC = 8
VPAD_TOT = 51200  # 50257 padded up to 400*128
VSH = VPAD_TOT // NC  # 6400 per-core vocab shard
OWN = N_TOK // NC  # 512 own tokens
IBLK = OWN // P  # 4 own row-blocks
KT = D // P  # 8 contraction tiles
OT = D // P  # 8 output-feature tiles
JB = N_TOK // 512  # 8 key strips of 512
JB2 = N_TOK // P  # 32 key tiles of 128
SCALE = 1.0 / 32.0  # 1/sqrt(D)

F32 = mybir.dt.float32
BF16 = mybir.dt.bfloat16
FP8 = mybir.dt.float8e4
I32 = mybir.dt.int32
WP_SCALE = 64.0
OUT_SCALE = 256.0

KV_K = P * OT * 512  # kT section elements in the kv exchange buffer
KV_V = P * D  # one V tile (128 tokens x 1024)
KV_ELEMS = KV_K + IBLK * KV_V

# logits v-strips within the 6400-wide shard: 12 x 512 + 1 x 256
VSTRIPS = [(i * 512, 512) for i in range(12)] + [(6144, 256)]


def build(nc: bass.Bass):
    qtok = nc.dram_tensor("qtok", [OWN], I32, kind="ExternalInput")
    E = nc.dram_tensor("E", [VOCAB, D], BF16, kind="ExternalInput")
    WqT = nc.dram_tensor("WqT", [D, D], BF16, kind="ExternalInput")
    WkT = nc.dram_tensor("WkT", [D, D], BF16, kind="ExternalInput")
    WvT = nc.dram_tensor("WvT", [D, D], BF16, kind="ExternalInput")
    bq = nc.dram_tensor("bq", [D], F32, kind="ExternalInput")
    bk = nc.dram_tensor("bk", [D], F32, kind="ExternalInput")
    Wp8 = nc.dram_tensor("Wp8", [D, VSH], FP8, kind="ExternalInput")
    # scol[h, jt, p] = jt*128 + p - (c*512 + h*256): causal mask thresholds
    scol = nc.dram_tensor("scol", [2 * JB2 * P], F32, kind="ExternalInput")
    logits = nc.dram_tensor("logits", [N_TOK, VSH], BF16, kind="ExternalOutput")

    with tile.TileContext(nc) as tc:
        with (
            tc.tile_pool(name="const", bufs=1) as const,
            tc.tile_pool(name="dram", bufs=1, space="DRAM") as dram,
        ):
            ident = const.tile([P, P], BF16)
            make_identity(nc, ident[:])

            bq_t = const.tile([P, OT], F32)
            nc.sync.dma_start(bq_t[:], bq.ap().rearrange("(ot p) -> p ot", p=P))
            bk_t = const.tile([P, OT], F32)
            nc.sync.dma_start(bk_t[:], bk.ap().rearrange("(ot p) -> p ot", p=P))

            scol_sb = const.tile([P, 2, JB2], F32)
            nc.sync.dma_start(
                scol_sb[:], scol.ap().rearrange("(h jt p) -> p h jt", p=P, h=2)
            )
            ones_col = const.tile([P, 1], BF16)
            nc.vector.memset(ones_col[:], 1.0)

            jidx0 = const.tile([P, 512], F32)

            qtok_sb = const.tile([P, OWN // P], I32)
            nc.sync.dma_start(qtok_sb[:], qtok.ap().rearrange("(g p) -> p g", p=P))

            # DRAM scratch for collectives
            kv_send = dram.tile([KV_ELEMS], FP8)
            kvg = dram.tile([NC, KV_ELEMS], FP8, addr_space="Shared")
            rrow_d = dram.tile([OWN], F32)
            oTb = [dram.tile([P, KT, P], FP8, name=f"oTb{q}") for q in range(IBLK)]
            gat = [
                dram.tile(
                    [NC, P, KT, P], FP8, name=f"gat{q}", addr_space="Shared"
                )
                for q in range(IBLK)
            ]

            # ---------------- phase QKV: own tokens only --------------------
            qT_pool = tc.alloc_tile_pool(name="qT_keep", bufs=1)
            qT = qT_pool.tile([P, OT, OWN], BF16)
            kT_pool = tc.alloc_tile_pool(name="kT_keep", bufs=1)
            kT_all = kT_pool.tile([P, OT, N_TOK], FP8)
            with (
                tc.tile_pool(name="sbw", bufs=1) as sbw,
                tc.tile_pool(name="sbq", bufs=2) as sbq,
                tc.tile_pool(name="psq_tr", bufs=2, space="PSUM") as psq_tr,
                tc.tile_pool(name="psq_pp", bufs=2, space="PSUM") as psq_pp,
                tc.tile_pool(name="psq_pv", bufs=2, space="PSUM") as psq_pv,
            ):
                ji = sbw.tile([P, 512], I32, tag="ji")
                nc.gpsimd.iota(ji[:], pattern=[[1, 512]], base=0, channel_multiplier=0)
                nc.vector.tensor_copy(out=jidx0[:], in_=ji[:])

                wq_b = sbw.tile([P, KT, D], BF16, tag="wq")
                nc.sync.dma_start(
                    wq_b[:], WqT.ap().rearrange("(kt p) o -> p kt o", p=P)
                )
                wk_b = sbw.tile([P, KT, D], BF16, tag="wk")
                nc.sync.dma_start(
                    wk_b[:], WkT.ap().rearrange("(kt p) o -> p kt o", p=P)
                )
                wv_b = sbw.tile([P, KT, D], BF16, tag="wv")
                nc.sync.dma_start(
                    wv_b[:], WvT.ap().rearrange("(kt p) o -> p kt o", p=P)
                )

                # gather own embeddings + transpose -> xT [P, KT, OWN]
                xT = sbw.tile([P, KT, OWN], BF16, tag="xT")
                for g in range(IBLK):
                    xg = sbq.tile([P, D], BF16, tag="xg")
                    nc.gpsimd.indirect_dma_start(
                        out=xg[:],
                        out_offset=None,
                        in_=E.ap(),
                        in_offset=bass.IndirectOffsetOnAxis(
                            ap=qtok_sb[:, g : g + 1], axis=0
                        ),
                    )
                    for kt in range(KT):
                        pst = psq_tr.tile([P, P], BF16, tag="ptr")
                        nc.tensor.transpose(
                            pst[:], xg[:, kt * P : (kt + 1) * P], ident[:]
                        )
                        nc.vector.tensor_copy(
                            out=xT[:, kt, g * P : (g + 1) * P], in_=pst[:]
                        )

                # K^T for own tokens -> kv_send[0:KV_K]
                kT_own = sbw.tile([P, OT, OWN], FP8, tag="kT_own")
                for ot in range(OT):
                    pk = psq_pp.tile([P, OWN], F32, tag="pp")
                    for kt in range(KT):
                        nc.tensor.matmul(
                            pk[:],
                            lhsT=wk_b[:, kt, ot * P : (ot + 1) * P],
                            rhs=xT[:, kt, :],
                            start=(kt == 0),
                            stop=(kt == KT - 1),
                        )
                    nc.vector.tensor_scalar(
                        out=kT_own[:, ot, :],
                        in0=pk[:],
                        scalar1=bk_t[:, ot : ot + 1],
                        scalar2=16.0,
                        op0=mybir.AluOpType.add,
                        op1=mybir.AluOpType.mult,
                    )
                nc.sync.dma_start(
                    kv_send[0:KV_K].rearrange(
                        "(p ot j) -> p ot j", p=P, ot=OT
                    ),
                    kT_own[:],
                )

                # V for own tokens (NO bias - folded into host bias_row)
                for tb in range(IBLK):
                    pv = psq_pv.tile([P, D], F32, tag="pv")
                    for half in range(2):
                        for kt in range(KT):
                            nc.tensor.matmul(
                                pv[:, half * 512 : (half + 1) * 512],
                                lhsT=xT[:, kt, tb * P : (tb + 1) * P],
                                rhs=wv_b[:, kt, half * 512 : (half + 1) * 512],
                                start=(kt == 0),
                                stop=(kt == KT - 1),
                            )
                    ve = sbq.tile([P, D], FP8, tag="ve")
                    nc.vector.tensor_scalar(
                        out=ve[:],
                        in0=pv[:],
                        scalar1=16.0,
                        scalar2=None,
                        op0=mybir.AluOpType.mult,
                    )
                    nc.sync.dma_start(
                        kv_send[
                            KV_K + tb * KV_V : KV_K + (tb + 1) * KV_V
                        ].rearrange("(p d) -> p d", p=P),
                        ve[:],
                    )

                nc.gpsimd.collective_compute(
                    "AllGather",
                    mybir.AluOpType.bypass,
                    replica_groups=[list(range(NC))],
                    ins=[kv_send.opt()],
                    outs=[kvg.opt()],
                )

                # Q^T (scaled by 1/sqrt(d)) kept in SBUF
                for ot in range(OT):
                    pp = psq_pp.tile([P, OWN], F32, tag="pp")
                    for kt in range(KT):
                        nc.tensor.matmul(
                            pp[:],
                            lhsT=wq_b[:, kt, ot * P : (ot + 1) * P],
                            rhs=xT[:, kt, :],
                            start=(kt == 0),
                            stop=(kt == KT - 1),
                        )
                    nc.vector.tensor_scalar(
                        out=qT[:, ot, :],
                        in0=pp[:],
                        scalar1=bq_t[:, ot : ot + 1],
                        scalar2=SCALE / 16.0,
                        op0=mybir.AluOpType.add,
                        op1=mybir.AluOpType.mult,
                    )


            # ---------------- load gathered K^T + Wp + bias -----------------
            wp_pool = tc.alloc_tile_pool(name="wp_keep", bufs=1)
            wp_b = wp_pool.tile([P, KT, VSH], FP8)
            nc.sync.dma_start(
                wp_b[:], Wp8.ap().rearrange("(kt p) v -> p kt v", p=P)
            )
            for r in range(NC):
                nc.sync.dma_start(
                    kT_all[:, :, r * 512 : (r + 1) * 512],
                    kvg[r, 0:KV_K].rearrange("(p ot j) -> p ot j", p=P, ot=OT),
                )

            # ---------------- phase attention (S^T layout, own rows) ---------
            with (
                tc.tile_pool(name="sbat", bufs=2) as sbat,
                tc.tile_pool(name="psat_sc", bufs=2, space="PSUM") as ps_sc,
                tc.tile_pool(name="psat_av", bufs=1, space="PSUM") as ps_av,
                tc.tile_pool(name="psat_sum", bufs=2, space="PSUM") as ps_sum,
            ):
                for half in range(2):
                    pav = ps_av.tile([P, KT, 256], F32, tag="av")
                    ssum = sbat.tile([1, 256], F32, tag="ssum")
                    nc.vector.memset(ssum[:], 0.0)
                    for jt in range(JB2):
                        ps = ps_sc.tile([P, 256], F32, tag="sc")
                        for ot in range(OT):
                            nc.tensor.matmul(
                                ps[:],
                                lhsT=kT_all[:, ot, jt * P : (jt + 1) * P],
                                rhs=qT[:, ot, half * 256 : (half + 1) * 256],
                                start=(ot == 0),
                                stop=(ot == OT - 1),
                            )
                        astr = sbat.tile([P, 256], BF16, tag="astr")
                        nc.scalar.activation(
                            astr[:], ps[:], mybir.ActivationFunctionType.Exp
                        )
                        # causal mask: keep exp where i_local >= scol (j <= i)
                        nc.vector.scalar_tensor_tensor(
                            out=astr[:],
                            in0=jidx0[:, 0:256],
                            scalar=scol_sb[:, half, jt : jt + 1],
                            in1=astr[:],
                            op0=mybir.AluOpType.is_ge,
                            op1=mybir.AluOpType.mult,
                        )
                        vj = sbat.tile([P, D], FP8, tag="vj")
                        r, tb = jt // IBLK, jt % IBLK
                        nc.sync.dma_start(
                            vj[:],
                            kvg[
                                r, KV_K + tb * KV_V : KV_K + (tb + 1) * KV_V
                            ].rearrange("(p d) -> p d", p=P),
                        )
                        for dc in range(KT):
                            nc.tensor.matmul(
                                pav[:, dc, :],
                                lhsT=vj[:, dc * P : (dc + 1) * P],
                                rhs=astr[:],
                                start=(jt == 0),
                                stop=(jt == JB2 - 1),
                            )
                        srow = ps_sum.tile([1, 256], F32, tag="srow")
                        nc.tensor.matmul(
                            srow[:],
                            lhsT=ones_col[:],
                            rhs=astr[:],
                            start=True,
                            stop=True,
                        )
                        nc.vector.tensor_add(
                            out=ssum[:], in0=ssum[:], in1=srow[:]
                        )
                    # normalize: oT = pav * (OUT_SCALE/16) / rowsum, in fp8
                    rr = sbat.tile([1, 256], F32, tag="rr")
                    nc.vector.reciprocal(rr[:], ssum[:])
                    rr2 = sbat.tile([1, 256], F32, tag="rr2")
                    nc.vector.tensor_scalar(
                        out=rr2[:],
                        in0=rr[:],
                        scalar1=OUT_SCALE / 16.0,
                        scalar2=None,
                        op0=mybir.AluOpType.mult,
                    )
                    nc.sync.dma_start(
                        rrow_d[half * 256 : (half + 1) * 256][None, :], rr2[:]
                    )
                    recip_bc = sbat.tile([P, 256], F32, tag="rbc")
                    nc.sync.dma_start(
                        recip_bc[:],
                        rrow_d[half * 256 : (half + 1) * 256][
                            None, :
                        ].to_broadcast([P, 256]),
                    )
                    oTn = sbat.tile([P, KT, 256], FP8, tag="oTn")
                    for dc in range(KT):
                        nc.vector.scalar_tensor_tensor(
                            out=oTn[:, dc, :],
                            in0=pav[:, dc, :],
                            scalar=1.0,
                            in1=recip_bc[:],
                            op0=mybir.AluOpType.mult,
                            op1=mybir.AluOpType.mult,
                        )
                    for qq in range(2):
                        q = half * 2 + qq
                        nc.sync.dma_start(
                            oTb[q][:], oTn[:, :, qq * P : (qq + 1) * P]
                        )
                        nc.gpsimd.collective_compute(
                            "AllGather",
                            mybir.AluOpType.bypass,
                            replica_groups=[list(range(NC))],
                            ins=[oTb[q].opt()],
                            outs=[gat[q].opt()],
                        )

            # ---------------- phase logits ----------------------------------
            with (
                tc.tile_pool(name="sblg", bufs=3) as sblg,
                tc.tile_pool(name="sblo", bufs=8) as sblo,
                tc.tile_pool(name="pslg", bufs=7, space="PSUM") as pslg,
            ):
                for q in range(IBLK):
                    for c in range(NC):
                        ibg = c * IBLK + q  # global row-block
                        lt8 = sblg.tile([P, KT, P], FP8, tag="lt8")
                        nc.sync.dma_start(lt8[:], gat[q][c, :, :, :])
                        for si, (v0, vw) in enumerate(VSTRIPS):
                            pl = pslg.tile([P, 512], F32, tag="lg")
                            for k2 in range(KT // 2):
                                nc.tensor.matmul(
                                    pl[:, :vw],
                                    lhsT=lt8[:, 2 * k2 : 2 * k2 + 2, :],
                                    rhs=wp_b[:, 2 * k2 : 2 * k2 + 2, v0 : v0 + vw],
                                    start=(k2 == 0),
                                    stop=(k2 == KT // 2 - 1),
                                    perf_mode=mybir.MatmulPerfMode.DoubleRow,
                                )
                            lo = sblo.tile([P, 512], BF16, tag="lo")
                            h = vw // 2
                            nc.scalar.activation(
                                lo[:, :h],
                                pl[:, :h],
                                mybir.ActivationFunctionType.Copy,
                            )
                            nc.vector.tensor_copy(
                                out=lo[:, h:vw], in_=pl[:, h:vw]
                            )
                            nc.gpsimd.dma_start(
                                logits.ap()[
                                    ibg * P : (ibg + 1) * P, v0 : v0 + vw
                                ],
                                lo[:, :vw],
                            )
            wp_pool.release()
            kT_pool.release()
            qT_pool.release()
    return nc


def _prep_inputs(inputs):
    """Host-side shard prep: slicing, transposes, padding, dtype pre-casts."""
    tokens = np.ascontiguousarray(np.asarray(inputs["tokens"]).astype(np.int32))
    E16 = np.asarray(inputs["E"], np.float32).astype(ml_dtypes.bfloat16)
    WqT = np.ascontiguousarray(
        np.asarray(inputs["Wq"], np.float32).T.astype(ml_dtypes.bfloat16)
    )
    WkT = np.ascontiguousarray(
        np.asarray(inputs["Wk"], np.float32).T.astype(ml_dtypes.bfloat16)
    )
    WvT = np.ascontiguousarray(
        np.asarray(inputs["Wv"], np.float32).T.astype(ml_dtypes.bfloat16)
    )
    Wp = np.asarray(inputs["Wp"], np.float32)
    bv = np.asarray(inputs["bv"], np.float32)
    WpT_pad = np.zeros((D, VPAD_TOT), np.float32)
    WpT_pad[:, :VOCAB] = Wp.T
    Wp8_full = (WpT_pad * WP_SCALE).astype(ml_dtypes.float8_e4m3)
    bias_full = np.zeros((VPAD_TOT,), np.float32)
    bias_full[:VOCAB] = np.asarray(inputs["bp"], np.float32) + Wp @ bv

    in_maps = []
    jt_idx = np.arange(JB2, dtype=np.float32)
    p_idx = np.arange(P, dtype=np.float32)
    for c in range(NC):
        # scol[h, jt, p] = jt*128 + p - (c*512 + h*256)
        scol = (
            jt_idx[None, :, None] * 128.0
            + p_idx[None, None, :]
            - (c * 512.0 + np.array([0.0, 256.0])[:, None, None])
        ).astype(np.float32)
        in_maps.append(
            {
                "qtok": np.ascontiguousarray(tokens[c * OWN : (c + 1) * OWN]),
                "E": E16,
                "WqT": WqT,
                "WkT": WkT,
                "WvT": WvT,
                "bq": np.asarray(inputs["bq"], np.float32),
                "bk": np.asarray(inputs["bk"], np.float32),
                "Wp8": np.ascontiguousarray(Wp8_full[:, c * VSH : (c + 1) * VSH]),
                "scol": np.ascontiguousarray(scol.ravel()),
            }
        )
    return in_maps, bias_full


def _run(inputs, trace=False):
    nc = bacc.Bacc(trn_type="TRN2", num_devices=NC, debug=False)
    build(nc)
    nc.compile()
    in_maps, bias_full = _prep_inputs(inputs)
    res = run_bass_kernel_spmd(
        nc, in_maps, core_ids=list(range(NC)), trace=trace
    )
    dq = 1.0 / (WP_SCALE * OUT_SCALE)
    out = np.concatenate(
        [
            np.asarray(res.results[c]["logits"], np.float32) * dq
            + bias_full[None, c * VSH : (c + 1) * VSH]
            for c in range(NC)
        ],
        axis=1,
    )[:, :VOCAB]
    return out, res


def kernel(**inputs) -> np.ndarray:
    out, _ = _run(inputs, trace=False)
    return out
